# revision 30
# baseline (speedup 1.0000x reference)
"""GATv2 (2-layer) Trainium2 Bass kernel, 8-core SPMD.

Dst-sharded graph parallel. See design notes at bottom of file.
"""

import os

import numpy as np

import concourse.bacc as bacc
import concourse.bass as bass
import concourse.mybir as mybir
from concourse.bass_utils import run_bass_kernel_spmd
from concourse.library_config import mlp
from concourse.tile import TileContext, add_dep_helper

F16 = mybir.dt.float16
F32 = mybir.dt.float32
AF = mybir.ActivationFunctionType
AX = mybir.AxisListType

NCORE = 8
BUCKETS = (4, 8, 16, 32, 64)
MASKVAL = -20000.0


# ---------------------------------------------------------------- structure
def _qperm(b):
    """Tile-partition permutation: the xr/table row of the node at slot
    position p lives at table row qperm[p] (xrt partition qperm[p]).
    For b>=16 this makes each per-j spread DMA source contiguous:
    positions {c*npchunk + j | c} <-> rows [j*C, (j+1)*C).  Identity for
    b<=8 (per-chunk spread path)."""
    q = np.arange(128)
    if b <= 8:
        return q
    C = min(b, 32)
    npchunk = 128 // b
    M = C * npchunk          # positions per sub (128 for b<=32, 64 for b=64)
    out = np.empty(128, np.int64)
    for base in range(0, 128, M):
        pl = np.arange(M)
        out[base + pl] = base + (pl % npchunk) * C + pl // npchunk
    return out


def _pperm(b):
    """Inverse of _qperm: position held at table row q."""
    q = _qperm(b)
    inv = np.empty(128, np.int64)
    inv[q] = np.arange(128)
    return inv


def build_plan(src, dst, n_nodes, ncore):
    npc = n_nodes // ncore
    deg = np.bincount(dst, minlength=n_nodes)
    assert deg.min() >= 1 and deg.max() <= BUCKETS[-1], (deg.min(), deg.max())
    bucket = np.full(n_nodes, BUCKETS[0], np.int64)
    for b in BUCKETS[1:]:
        bucket[deg > b // 2] = b
    core_of = np.arange(n_nodes) // npc

    ncap_b = {}
    for b in BUCKETS:
        cnt = max(((bucket == b) & (core_of == c)).sum() for c in range(ncore))
        ncap_b[b] = ((cnt + 127) // 128) * 128
    ncap = sum(ncap_b.values())
    ng = ncore * ncap
    gbase = ng // 2
    assert ng <= 65534, ng

    # tiles: (bucket, node offset within core's sorted order)
    tiles = []
    pos = 0
    for b in BUCKETS:
        for t in range(ncap_b[b] // 128):
            tiles.append((b, pos + t * 128))
        pos += ncap_b[b]
    totc = sum(b for b, _ in tiles)

    # per-core node order (sorted by bucket), -1 = dummy
    order = np.full((ncore, ncap), -1, np.int64)
    grow = np.full(n_nodes, -1, np.int64)   # global table row of node
    colmap = np.empty(ncap, np.int64)       # position -> table row (local)
    for (b, p0) in tiles:
        colmap[p0:p0 + 128] = p0 + _qperm(b)

    def _grow_assign(c):
        nodes = order[c]
        ok = nodes >= 0
        grow[nodes[ok]] = c * ncap + colmap[np.where(ok)[0]]

    for c in range(ncore):
        pos = 0
        for b in BUCKETS:
            nodes = np.where((bucket == b) & (core_of == c))[0]
            order[c, pos:pos + len(nodes)] = nodes
            pos += ncap_b[b]
        _grow_assign(c)

    # CSR of incoming edges by dst
    es = np.argsort(dst, kind="stable")
    ssrc = src[es]
    starts = np.zeros(n_nodes + 1, np.int64)
    np.cumsum(deg, out=starts[1:])

    # idx + mask per core; ensure each tile's last gather idx >= 0
    idx16 = np.zeros((ncore, totc * 128), np.int16)
    mask = np.zeros((ncore, 128, totc), np.float16)
    for c in range(ncore):
        # first fix node order so each tile's LAST real node can end >= 0
        tile_node_lists = []
        for (b, p0) in tiles:
            tile_node_lists.append(list(order[c, p0:p0 + 128]))
        for tl, (b, p0) in zip(tile_node_lists, tiles):
            last = tl[-1]
            if last < 0:
                continue  # dummy last -> idx 0, fine
            rows = grow[ssrc[starts[last]:starts[last] + deg[last]]] - gbase
            if deg[last] < b or (rows >= 0).any():
                continue  # pad slot last, or reorderable
            # swap with a node that can end non-negative
            for j in range(127):
                n2 = tl[j]
                if n2 < 0:
                    tl[j], tl[-1] = tl[-1], tl[j]
                    break
                r2 = grow[ssrc[starts[n2]:starts[n2] + deg[n2]]] - gbase
                if deg[n2] < b or (r2 >= 0).any():
                    tl[j], tl[-1] = tl[-1], tl[j]
                    break
            else:
                raise AssertionError("tile unfixable for trailing-negative")
        # rewrite order/grow after swaps
        for tl, (b, p0) in zip(tile_node_lists, tiles):
            order[c, p0:p0 + 128] = tl
        _grow_assign(c)

    for c in range(ncore):
        slot = 0
        for (b, p0) in tiles:
            for j in range(128):
                node = order[c, p0 + j]
                if node < 0:
                    slot += b  # dummy: idx 0, unmasked (finite junk)
                    continue
                d = deg[node]
                rows = (grow[ssrc[starts[node]:starts[node] + d]] - gbase)
                rows = np.sort(rows)  # negatives first, non-negatives last
                idx16[c, slot:slot + d] = rows.astype(np.int16)
                for s in range(slot + d, slot + b):
                    mask[c, s % 128, s // 128] = MASKVAL
                slot += b
        assert slot == totc * 128
        # verify per-tile trailing idx
        soff = 0
        for (b, p0) in tiles:
            assert idx16[c, soff + b * 128 - 1] >= 0
            soff += b * 128

    # wrap idx into the [16, n/16] layout, replicated across 128 partitions
    idxw = np.zeros((ncore, 128, totc * 8), np.int16)
    for c in range(ncore):
        w = idx16[c].reshape(totc * 8, 16).T  # idx i -> [i%16, i//16]
        idxw[c] = np.tile(w, (8, 1))

    return dict(deg=deg, bucket=bucket, ncap_b=ncap_b, ncap=ncap, ng=ng,
                gbase=gbase, tiles=tiles, totc=totc, nt=len(tiles),
                order=order, grow=grow, idxw=idxw, mask=mask, colmap=colmap)


def _patterns():
    """Agg one-hot patterns S_{b,k} [128, 32] f16, concatenated; plus offsets."""
    pats, offs = [], {}
    col = 0
    for b in BUCKETS:
        kd = (32 * b) // 128  # chunks per 32-node block
        offs[b] = col
        for k in range(kd):
            p = np.zeros((128, 32), np.float16)
            for q in range(128):
                p[q, (k * 128 + q) // b] = 1.0
            pats.append(p)
            col += 32
    return np.concatenate(pats, axis=1), offs


def _vmats():
    """V_b [128,128] f16 per bucket: hT col q = h row pperm_b[q], so the
    layer-2 table adopts the same qperm row order as layer 1's."""
    mats = []
    for b in BUCKETS:
        m = np.zeros((128, 128), np.float16)
        m[_pperm(b), np.arange(128)] = 1.0
        mats.append(m)
    return np.concatenate(mats, axis=1)


def _repmats():
    """E_b lhsT [128,128] f16 per bucket: out[p] = xs[p0(node(p))]; chunk-k
    invariant for pow2 buckets (no run straddle)."""
    mats, offs = [], {}
    col = 0
    for b in BUCKETS:
        m = np.zeros((128, 128), np.float16)
        for p in range(128):
            m[(p // b) * b, p] = 1.0  # lhsT[q, p]
        mats.append(m)
        offs[b] = col
        col += 128
    return np.concatenate(mats, axis=1), offs


# ---------------------------------------------------------------- weights
def prep_weights(W1_l, W1_r, b1_l, b1_r, a1, bias1, W2_l, W2_r, b2_l, b2_r,
                 a2, bias2):
    """Sign-permute features, fold a into tables; build packed weight mats."""
    p1 = np.argsort(a1 < 0, kind="stable")     # a1>=0 first
    n1p = int((a1 >= 0).sum())
    a1p = a1[p1]
    W1_lp, W1_rp = W1_l[:, p1], W1_r[:, p1]
    b1_lp, b1_rp = b1_l[p1], b1_r[p1]
    bias1p = bias1[p1]
    p2 = np.argsort(a2 < 0, kind="stable")
    n2p = int((a2 >= 0).sum())
    a2p = a2[p2]
    # W2 rows live in h-space -> permute rows by p1; columns by p2
    W2_lp, W2_rp = W2_l[p1][:, p2], W2_r[p1][:, p2]
    b2_lp, b2_rp = b2_l[p2], b2_r[p2]
    bias2p = bias2[p2]

    w1pack = np.concatenate([
        W1_lp * a1p[None, :], 1.5 * (W1_lp @ a1p)[:, None],
        W1_rp * a1p[None, :], 1.5 * (W1_rp @ a1p)[:, None]], axis=1)  # [128,130]
    b1pack = np.concatenate([
        b1_lp * a1p, [1.5 * (b1_lp @ a1p)],
        b1_rp * a1p, [1.5 * (b1_rp @ a1p)]])                          # [130]
    w2pack = np.concatenate([
        W2_lp * a2p[None, :], 1.5 * (W2_lp @ a2p)[:, None],
        W2_rp * a2p[None, :], 1.5 * (W2_rp @ a2p)[:, None]], axis=1)  # [64,34]
    b2pack = np.concatenate([
        b2_lp * a2p, [1.5 * (b2_lp @ a2p)],
        b2_rp * a2p, [1.5 * (b2_rp @ a2p)]])                          # [34]
    inv1 = (1.0 / a1p).astype(np.float32)
    inv2 = (1.0 / a2p).astype(np.float32)
    return dict(p1=p1, p2=p2, n1p=n1p, n2p=n2p, w1pack=w1pack, b1pack=b1pack,
                w2pack=w2pack, b2pack=b2pack, inv1=inv1, inv2=inv2,
                bias1p=bias1p.astype(np.float32), bias2p=bias2p.astype(np.float32))


# ---------------------------------------------------------------- device
def build_program(plan, wp, ncore):
    ncap, nt, totc, gbase = plan["ncap"], plan["nt"], plan["totc"], plan["gbase"]
    tiles = plan["tiles"]
    ng = plan["ng"]
    patv, patoffs = _patterns()
    edv, edoffs = _repmats()
    npat = patv.shape[1]

    nc = bacc.Bacc("TRN2", num_swdge_queues=4,
                   dynamic_dma_scratch_size=2 ** 15)
    XT = nc.declare_dram_parameter("XT", [128, ncap], F16, isOutput=False)
    W1P = nc.declare_dram_parameter("W1P", [128, 130], F16, isOutput=False)
    B1P = nc.declare_dram_parameter("B1P", [128, 130], F16, isOutput=False)
    W2P = nc.declare_dram_parameter("W2P", [64, 34], F16, isOutput=False)
    B2P = nc.declare_dram_parameter("B2P", [128, 34], F16, isOutput=False)
    IDX = nc.declare_dram_parameter("IDX", [128, totc * 8], mybir.dt.int16, isOutput=False)
    MASK = nc.declare_dram_parameter("MASK", [128, totc], F16, isOutput=False)
    PATS = nc.declare_dram_parameter("PATS", [128, npat], F16, isOutput=False)
    EDS = nc.declare_dram_parameter("EDS", [128, len(BUCKETS) * 128], F16, isOutput=False)
    VPERM = nc.declare_dram_parameter("VPERM", [128, len(BUCKETS) * 128], F16, isOutput=False)
    EYE = nc.declare_dram_parameter("EYE", [128, 128], F16, isOutput=False)
    INV1 = nc.declare_dram_parameter("INV1", [128, 64], F32, isOutput=False)
    BS1 = nc.declare_dram_parameter("BS1", [128, 64], F32, isOutput=False)
    INV2 = nc.declare_dram_parameter("INV2", [128, 16], F32, isOutput=False)
    BS2 = nc.declare_dram_parameter("BS2", [128, 16], F32, isOutput=False)
    OUT = nc.declare_dram_parameter("OUT", [ncap, 16], F32, isOutput=True)

    T1s = nc.dram_tensor("T1s", [ncap, 128], F16)
    shared = "Shared" if ncore > 4 else "Local"
    T1f = nc.dram_tensor("T1f", [ng, 128], F16, addr_space=shared)
    T2s = nc.dram_tensor("T2s", [ncap, 128], F16)
    T2f = nc.dram_tensor("T2f", [ng, 128], F16, addr_space=shared)
    # Local copies of the gathered tables: SWDGE random reads from the
    # Shared address space run ~3x slower than from Local DRAM, so copy
    # once (sequential, cheap) and gather from the Local replica.
    T1fl = nc.dram_tensor("T1fl", [ng, 128], F16)
    T2fl = nc.dram_tensor("T2fl", [ng, 128], F16)

    with TileContext(nc) as tc:
        nc.gpsimd.load_library(mlp)
        with tc.tile_pool(name="const", bufs=1) as cpool, \
             tc.tile_pool(name="work", bufs=5) as pool, \
             tc.tile_pool(name="zpool", bufs=6) as zpool, \
             tc.tile_pool(name="pz", bufs=3, space="PSUM") as pzpool, \
             tc.tile_pool(name="pa", bufs=2, space="PSUM") as papool:

            # persistent constants
            w1p = cpool.tile([128, 130], F16); nc.sync.dma_start(w1p[:], W1P[:])
            b1p = cpool.tile([128, 130], F16); nc.sync.dma_start(b1p[:], B1P[:])
            w2p = cpool.tile([64, 34], F16);   nc.sync.dma_start(w2p[:], W2P[:])
            b2p = cpool.tile([128, 34], F16);  nc.sync.dma_start(b2p[:], B2P[:])
            idxs = cpool.tile([128, totc * 8], mybir.dt.int16)
            nc.sync.dma_start(idxs[:], IDX[:])
            maskt = cpool.tile([128, totc], F16); nc.sync.dma_start(maskt[:], MASK[:])
            pats = cpool.tile([128, npat], F16); nc.sync.dma_start(pats[:], PATS[:])
            eds = cpool.tile([128, len(BUCKETS) * 128], F16)
            nc.sync.dma_start(eds[:], EDS[:])
            vperm = cpool.tile([128, len(BUCKETS) * 128], F16)
            nc.sync.dma_start(vperm[:], VPERM[:])
            eye = cpool.tile([128, 128], F16); nc.sync.dma_start(eye[:], EYE[:])
            inv1 = cpool.tile([128, 64], F32); nc.sync.dma_start(inv1[:], INV1[:])
            bs1 = cpool.tile([128, 64], F32); nc.sync.dma_start(bs1[:], BS1[:])
            inv2 = cpool.tile([128, 16], F32); nc.sync.dma_start(inv2[:], INV2[:])
            bs2 = cpool.tile([128, 16], F32); nc.sync.dma_start(bs2[:], BS2[:])
            xrt = cpool.tile([128, nt * 65], F16)      # x_r''' per sorted node
            h2rt = cpool.tile([128, nt * 17], F16)     # layer-2 r-side per node
            xs2 = [cpool.tile([128, 32 * 65], F16, name=f"xs{i}")
                   for i in range(4)]
            stg2 = [cpool.tile([128, 128], F16, name=f"stg{i}")
                    for i in range(3)]
            for t_ in xs2 + stg2:
                nc.gpsimd.memset(t_[:], 0.0)
            nc.gpsimd.memset(xrt[:], 0.0)
            nc.gpsimd.memset(h2rt[:], 0.0)

            # ---------------- phase A: layer-1 tables ----------------
            xrt_w, h2rt_w = [], []
            ni_regs = {cc: nc.gpsimd.to_reg(cc * 128)
                       for cc in sorted({min(b, 32) for b in BUCKETS})}
            for t in range(nt):
                xtc = pool.tile([128, 128], F16, tag="xtc")
                nc.sync.dma_start(xtc[:], XT[:, t * 128:(t + 1) * 128])
                psA = papool.tile([128, 130], F32, tag="tmp")
                stg = stg2[t % 3]
                nc.tensor.matmul(psA[:], xtc[:], w1p[:], start=True, stop=True)
                nc.vector.tensor_add(stg[:, 0:65], psA[:, 0:65], b1p[:, 0:65])
                nc.vector.tensor_scalar(stg[:, 65:66], psA[:, 64:65], 0.0, 1.0,
                                        mybir.AluOpType.mult, mybir.AluOpType.add)
                xrt_w.append(nc.vector.tensor_add(
                    xrt[:, t * 65:(t + 1) * 65],
                    psA[:, 65:130], b1p[:, 65:130]).ins)
                nc.sync.dma_start(T1s[t * 128:(t + 1) * 128, :], stg[:])
            if not os.environ.get("GAT_SKIP_CC"):
                nc.gpsimd.collective_compute(
                    "AllGather", mybir.AluOpType.bypass,
                    replica_groups=[list(range(ncore))],
                    ins=[T1s[:]], outs=[T1f[:]])
            else:
                nc.sync.dma_start(T1f[0:ncap, :], T1s[:, :])
            hg = ng // 2
            nc.sync.dma_start(T1fl[0:hg, :], T1f[0:hg, :])
            nc.scalar.dma_start(T1fl[hg:, :], T1f[hg:, :])

            # ---------------- phase C/E: per-layer edge phases ----------------
            qrr = [0]  # SWDGE queue round-robin

            def layer(F, Tf, xr_src, xr_w, n_pos, emit, xr_dep=None):
                K = 3      # pass2 (softmax+agg) lags pass1 by K subs

                def pass2(w):
                    C, b, prow = w["C"], w["b"], w["prow"]
                    az, pm, zt, psa = w["az"], w["pm"], w["zt"], w["psa"]
                    rp = pool.tile([128, 32], F32, tag="rp")
                    rm = pool.tile([128, 32], F32, tag="rm")
                    nc.vector.reduce_sum(rp[:, 0:C], az[:, 0:C, 0:n_pos],
                                         axis=AX.X)
                    nc.vector.reduce_sum(rm[:, 0:C], az[:, 0:C, n_pos:F],
                                         axis=AX.X)
                    u = pool.tile([128, 32], F32, tag="u")
                    nc.vector.tensor_sub(u[:, 0:C], rp[:, 0:C], rm[:, 0:C])
                    nc.vector.tensor_add(u[:, 0:C], u[:, 0:C], pm[:, 0:C])
                    ex = pool.tile([128, 32], F16, tag="ex")
                    nc.scalar.activation(ex[:, 0:C], u[:, 0:C], AF.Exp,
                                         scale=0.4)
                    sv = pool.tile([128, 32 * 32], F16, tag="sv")
                    kd = (32 * b) // 128      # chunks per 32-node block
                    nblk = C // kd
                    pf = pats[:, :]
                    pat_ap = bass.AP(pf.tensor, pf.offset + patoffs[b],
                                     [[pf.ap[0][0], 128], [0, nblk],
                                      [1, kd * 32]])
                    svv = sv.rearrange("p (n m) -> p n m", m=kd * 32)[:, 0:nblk, :]
                    exv = ex.rearrange("p (n k) -> p n k", k=kd)[:, 0:nblk, :]
                    exb = exv.unsqueeze(3).broadcast_to([128, nblk, kd, 32])
                    nc.vector.tensor_mul(
                        svv.rearrange("p n (k m) -> p n k m", m=32), pat_ap, exb)
                    for c in range(C):
                        blk = prow // 32 + c // kd
                        nc.tensor.matmul(
                            psa[32 * blk:32 * blk + 32, :],
                            sv[:, c * 32:(c + 1) * 32],
                            zt[:, c, 0:F + 2],
                            start=(c % kd == 0), stop=(c % kd == kd - 1),
                            tile_position=(0, 32 * blk), skip_group_check=True)
                    if w["last"]:
                        emit(w["ti"], psa)

                pend = []
                soff = 0   # chunk offset
                for ti, (b, p0) in enumerate(tiles):
                    xs = xs2[ti % 4]
                    subs = [(0, b)] if b <= 32 else [(0, 32), (64, 32)]
                    psa = papool.tile([128, F + 2], F32, tag="psa", bufs=3)
                    for si, (prow, C) in enumerate(subs):
                        zt = zpool.tile([128, 32, 128], F16, tag="zt")
                        if os.environ.get("GAT_SKIP_GATHER"):
                            nc.sync.dma_start(zt[:, 0:C, :],
                                              Tf[0:128, :].unsqueeze(1).broadcast_to([128, C, 128]))
                        else:
                            GMAX = 8  # chunks per gather (<=1024 idxs, ucode limit)
                            for g0 in range(0, C, GMAX):
                                g1 = min(g0 + GMAX, C)
                                nig = (g1 - g0) * 128
                                nc.gpsimd.dma_gather(
                                    zt[:, g0:g1, :], Tf[gbase:, :],
                                    idxs[:, (soff + g0) * 8:(soff + g1) * 8],
                                    nig, nig, 128, queue_num=qrr[0] % 4)
                                qrr[0] += 1
                        # spread xr rows: node j of chunk c at partition j*b.
                        # For b>=16 the table is qperm-ordered so slot-j's
                        # nodes sit at contiguous rows [j*C, (j+1)*C).
                        npchunk = 128 // b
                        xsf = xs[:, :]
                        sps = xsf.ap[0][0]
                        if b >= 16:
                            for j in range(npchunk):
                                src = xr_src[prow + j * C:prow + (j + 1) * C,
                                             ti * (F + 1):(ti + 1) * (F + 1)]
                                dst = xs[j * b:j * b + 1, 0:C * (F + 1)]
                                eng = nc.sync if j % 2 == 0 else nc.scalar
                                iv = eng.dma_start(dst, src)
                                if xr_dep is not None:
                                    add_dep_helper(iv.ins, xr_dep[ti], sync=True,
                                                   reason="spread reads xr table")
                        else:
                            for c in range(C):
                                src = xr_src[prow + c * npchunk:prow + (c + 1) * npchunk,
                                             ti * (F + 1):(ti + 1) * (F + 1)]
                                dst = bass.AP(xsf.tensor, xsf.offset + c * (F + 1),
                                              [[sps * b, npchunk], [1, F + 1]])
                                eng = nc.sync if c % 2 == 0 else nc.scalar
                                iv = eng.dma_start(dst, src)
                                if xr_dep is not None:
                                    add_dep_helper(iv.ins, xr_dep[ti], sync=True,
                                                   reason="spread reads xr table")
                        # z' psum: vals + q separately (bank-aligned)
                        pzq = papool.tile([128, 32], F32, tag="tmp")
                        cpg = 512 // F
                        xsv = xs[:, 0:C * (F + 1)].rearrange("p (c f) -> p c f", f=F + 1)
                        az = pool.tile([128, 32, F], F16, tag="az")
                        for c0 in range(0, C, cpg):
                            c1 = min(c0 + cpg, C)
                            pz = pzpool.tile([128, cpg * F], F32, tag="pz")
                            nc.tensor.matmul(pz[:, 0:(c1 - c0) * F], eye[:],
                                             zt[:, c0:c1, 0:F],
                                             start=True, stop=False)
                            nc.tensor.matmul(pz[:, 0:(c1 - c0) * F],
                                             eds[:, edoffs[b]:edoffs[b] + 128],
                                             xsv[:, c0:c1, 0:F],
                                             start=False, stop=True)
                            pzv = pz.rearrange("p (c f) -> p c f", f=F)[:, 0:c1 - c0, :]
                            nc.scalar.activation(az[:, c0:c1, :], pzv[:, :, :], AF.Abs)
                        ztf = zt[:, :, :]
                        zqcol = bass.AP(ztf.tensor, ztf.offset + F,
                                        [[ztf.ap[0][0], 128], [128, C]])
                        nc.tensor.matmul(pzq[:, 0:C], eye[:], zqcol,
                                         start=True, stop=False)
                        xqcol = bass.AP(xsf.tensor, xsf.offset + F,
                                        [[sps, 128], [F + 1, C]])
                        nc.tensor.matmul(pzq[:, 0:C],
                                         eds[:, edoffs[b]:edoffs[b] + 128],
                                         xqcol, start=False, stop=True)
                        pm = pool.tile([128, 32], F32, tag="pm")
                        nc.vector.tensor_add(pm[:, 0:C], pzq[:, 0:C],
                                             maskt[:, soff:soff + C])
                        pend.append(dict(C=C, b=b, prow=prow, az=az, pm=pm,
                                         zt=zt, psa=psa, ti=ti,
                                         last=(si == len(subs) - 1)))
                        soff += C
                        if len(pend) > K:
                            pass2(pend.pop(0))
                for w in pend:
                    pass2(w)

            # layer-1 epilogue: h, transpose, layer-2 tables
            def emit1(ti, psa):
                stg = stg2[ti % 3]
                rden = pool.tile([128, 1], F32, tag="rden")
                nc.vector.reciprocal(rden[:], psa[:, 65:66])
                h1 = pool.tile([128, 64], F32, tag="h1")
                nc.vector.tensor_scalar_mul(h1[:], psa[:, 0:64], rden[:])
                nc.vector.tensor_mul(h1[:], h1[:], inv1[:])
                nc.vector.tensor_add(h1[:], h1[:], bs1[:])
                h = pool.tile([128, 64], F16, tag="h")
                nc.scalar.activation(h[:], h1[:], AF.Relu)
                ptp = papool.tile([64, 128], F16, tag="tmp")
                bt = tiles[ti][0]
                nc.tensor.transpose(ptp[:], h[:],
                                    vperm[:, edoffs[bt]:edoffs[bt] + 128])
                hT = pool.tile([64, 128], F16, tag="hT")
                nc.scalar.copy(hT[:], ptp[:])
                ps2 = papool.tile([128, 34], F32, tag="tmp")
                nc.tensor.matmul(ps2[:], hT[:], w2p[:], start=True, stop=True)
                nc.vector.tensor_add(stg[:, 0:17], ps2[:, 0:17], b2p[:, 0:17])
                nc.vector.tensor_scalar(stg[:, 17:18], ps2[:, 16:17], 0.0, 1.0,
                                        mybir.AluOpType.mult, mybir.AluOpType.add)
                h2rt_w.append(nc.vector.tensor_add(
                    h2rt[:, ti * 17:(ti + 1) * 17],
                    ps2[:, 17:34], b2p[:, 17:34]).ins)
                nc.sync.dma_start(T2s[ti * 128:(ti + 1) * 128, :], stg[:, :])

            def emit2(ti, psa):
                rden = pool.tile([128, 1], F32, tag="rden")
                nc.vector.reciprocal(rden[:], psa[:, 17:18])
                o1 = pool.tile([128, 16], F32, tag="o1")
                nc.vector.tensor_scalar_mul(o1[:], psa[:, 0:16], rden[:])
                nc.vector.tensor_mul(o1[:], o1[:], inv2[:])
                nc.vector.tensor_add(o1[:], o1[:], bs2[:])
                nc.sync.dma_start(OUT[ti * 128:(ti + 1) * 128, :], o1[:])

            layer(64, T1fl, xrt, 65, wp["n1p"], emit1, xr_dep=xrt_w)
            if not os.environ.get("GAT_SKIP_CC"):
                nc.gpsimd.collective_compute(
                    "AllGather", mybir.AluOpType.bypass,
                    replica_groups=[list(range(ncore))],
                    ins=[T2s[:]], outs=[T2f[:]])
            else:
                nc.sync.dma_start(T2f[0:ncap, :], T2s[:, :])
            nc.sync.dma_start(T2fl[0:hg, :], T2f[0:hg, :])
            nc.scalar.dma_start(T2fl[hg:, :], T2f[hg:, :])
            layer(16, T2fl, h2rt, 17, wp["n2p"], emit2, xr_dep=h2rt_w)

    nc.compile()
    return nc


# ---------------------------------------------------------------- host entry
def kernel(x, edge_index, W1_l, W1_r, b1_l, b1_r, a1, bias1,
           W2_l, W2_r, b2_l, b2_r, a2, bias2, _run=None, _ncore=NCORE):
    x = np.asarray(x, np.float32)
    ei = np.asarray(edge_index)
    n = x.shape[0]
    loop = np.arange(n, dtype=ei.dtype)
    src = np.concatenate([np.asarray(ei[0]), loop]).astype(np.int64)
    dst = np.concatenate([np.asarray(ei[1]), loop]).astype(np.int64)

    plan = build_plan(src, dst, n, _ncore)
    wp = prep_weights(*[np.asarray(a, np.float32) for a in
                        (W1_l, W1_r, b1_l, b1_r, a1, bias1,
                         W2_l, W2_r, b2_l, b2_r, a2, bias2)])
    nc = build_program(plan, wp, _ncore)

    patv, _ = _patterns()
    edvv, _ = _repmats()
    vmv = _vmats()
    in_maps = []
    for c in range(_ncore):
        xt = np.zeros((128, plan["ncap"]), np.float16)
        ordc = plan["order"][c]
        valid = ordc >= 0
        xt[:, plan["colmap"][np.where(valid)[0]]] = \
            x[ordc[valid]].T.astype(np.float16)
        in_maps.append({
            "XT": xt,
            "W1P": wp["w1pack"].astype(np.float16),
            "B1P": np.tile(wp["b1pack"][None, :].astype(np.float16), (128, 1)),
            "W2P": wp["w2pack"].astype(np.float16),
            "B2P": np.tile(wp["b2pack"][None, :].astype(np.float16), (128, 1)),
            "IDX": plan["idxw"][c],
            "MASK": plan["mask"][c],
            "PATS": patv,
            "EDS": edvv,
            "VPERM": vmv,
            "EYE": np.eye(128, dtype=np.float16),
            "INV1": np.tile(wp["inv1"][None, :], (128, 1)),
            "BS1": np.tile(wp["bias1p"][None, :], (128, 1)),
            "INV2": np.tile(wp["inv2"][None, :], (128, 1)),
            "BS2": np.tile(wp["bias2p"][None, :], (128, 1)),
        })

    if _run is None:
        import time as _time
        res = run_bass_kernel_spmd(nc, in_maps, list(range(_ncore)))
        outs = [r["OUT"] for r in res.results]
        if os.environ.get("GAT_TRACE"):
            ts = []
            for _ in range(3):
                t0 = _time.time()
                run_bass_kernel_spmd(nc, in_maps, list(range(_ncore)))
                ts.append(_time.time() - t0)
            # min wall of a cached re-dispatch (includes host<->device I/O)
            print(f"HW exec time: {int(min(ts) * 1e9)} ns (e2e dispatch wall, "
                  f"runs: {[f'{t:.3f}s' for t in ts]})")
    else:
        outs = _run(nc, in_maps)   # test hook: returns list of OUT per core

    # unshard: rows sorted-order per core -> natural; cols: undo p2
    out = np.zeros((n, 16), np.float32)
    for c in range(_ncore):
        ordc = plan["order"][c]
        valid = ordc >= 0
        out[ordc[valid]] = outs[c][np.where(valid)[0]]
    inv_p2 = np.argsort(wp["p2"])
    return out[:, inv_p2].astype(np.float32)



# revision 31
# speedup vs baseline: 1.0086x; 1.0086x over previous
"""GATv2 (2-layer) Trainium2 Bass kernel, 8-core SPMD.

Dst-sharded graph parallel. See design notes at bottom of file.
"""

import os

import numpy as np

import concourse.bacc as bacc
import concourse.bass as bass
import concourse.mybir as mybir
from concourse.bass_utils import run_bass_kernel_spmd
from concourse.library_config import mlp
from concourse.tile import TileContext, add_dep_helper

F16 = mybir.dt.float16
F32 = mybir.dt.float32
AF = mybir.ActivationFunctionType
AX = mybir.AxisListType

NCORE = 8
BUCKETS = (4, 8, 16, 32, 64)
MASKVAL = -20000.0


# ---------------------------------------------------------------- structure
def _qperm(b):
    """Tile-partition permutation: the xr/table row of the node at slot
    position p lives at table row qperm[p] (xrt partition qperm[p]).
    For b>=16 this makes each per-j spread DMA source contiguous:
    positions {c*npchunk + j | c} <-> rows [j*C, (j+1)*C).  Identity for
    b<=8 (per-chunk spread path)."""
    q = np.arange(128)
    if b <= 8:
        return q
    C = min(b, 32)
    npchunk = 128 // b
    M = C * npchunk          # positions per sub (128 for b<=32, 64 for b=64)
    out = np.empty(128, np.int64)
    for base in range(0, 128, M):
        pl = np.arange(M)
        out[base + pl] = base + (pl % npchunk) * C + pl // npchunk
    return out


def _pperm(b):
    """Inverse of _qperm: position held at table row q."""
    q = _qperm(b)
    inv = np.empty(128, np.int64)
    inv[q] = np.arange(128)
    return inv


def build_plan(src, dst, n_nodes, ncore):
    npc = n_nodes // ncore
    deg = np.bincount(dst, minlength=n_nodes)
    assert deg.min() >= 1 and deg.max() <= BUCKETS[-1], (deg.min(), deg.max())
    bucket = np.full(n_nodes, BUCKETS[0], np.int64)
    for b in BUCKETS[1:]:
        bucket[deg > b // 2] = b
    core_of = np.arange(n_nodes) // npc

    ncap_b = {}
    for b in BUCKETS:
        cnt = max(((bucket == b) & (core_of == c)).sum() for c in range(ncore))
        ncap_b[b] = ((cnt + 127) // 128) * 128
    ncap = sum(ncap_b.values())
    ng = ncore * ncap
    gbase = ng // 2
    assert ng <= 65534, ng

    # tiles: (bucket, node offset within core's sorted order)
    tiles = []
    pos = 0
    for b in BUCKETS:
        for t in range(ncap_b[b] // 128):
            tiles.append((b, pos + t * 128))
        pos += ncap_b[b]
    totc = sum(b for b, _ in tiles)

    # per-core node order (sorted by bucket), -1 = dummy
    order = np.full((ncore, ncap), -1, np.int64)
    grow = np.full(n_nodes, -1, np.int64)   # global table row of node
    colmap = np.empty(ncap, np.int64)       # position -> table row (local)
    for (b, p0) in tiles:
        colmap[p0:p0 + 128] = p0 + _qperm(b)

    def _grow_assign(c):
        nodes = order[c]
        ok = nodes >= 0
        grow[nodes[ok]] = c * ncap + colmap[np.where(ok)[0]]

    for c in range(ncore):
        pos = 0
        for b in BUCKETS:
            nodes = np.where((bucket == b) & (core_of == c))[0]
            order[c, pos:pos + len(nodes)] = nodes
            pos += ncap_b[b]
        _grow_assign(c)

    # CSR of incoming edges by dst
    es = np.argsort(dst, kind="stable")
    ssrc = src[es]
    starts = np.zeros(n_nodes + 1, np.int64)
    np.cumsum(deg, out=starts[1:])

    # idx + mask per core; ensure each tile's last gather idx >= 0
    idx16 = np.zeros((ncore, totc * 128), np.int16)
    mask = np.zeros((ncore, 128, totc), np.float16)
    for c in range(ncore):
        # first fix node order so each tile's LAST real node can end >= 0
        tile_node_lists = []
        for (b, p0) in tiles:
            tile_node_lists.append(list(order[c, p0:p0 + 128]))
        for tl, (b, p0) in zip(tile_node_lists, tiles):
            last = tl[-1]
            if last < 0:
                continue  # dummy last -> idx 0, fine
            rows = grow[ssrc[starts[last]:starts[last] + deg[last]]] - gbase
            if deg[last] < b or (rows >= 0).any():
                continue  # pad slot last, or reorderable
            # swap with a node that can end non-negative
            for j in range(127):
                n2 = tl[j]
                if n2 < 0:
                    tl[j], tl[-1] = tl[-1], tl[j]
                    break
                r2 = grow[ssrc[starts[n2]:starts[n2] + deg[n2]]] - gbase
                if deg[n2] < b or (r2 >= 0).any():
                    tl[j], tl[-1] = tl[-1], tl[j]
                    break
            else:
                raise AssertionError("tile unfixable for trailing-negative")
        # rewrite order/grow after swaps
        for tl, (b, p0) in zip(tile_node_lists, tiles):
            order[c, p0:p0 + 128] = tl
        _grow_assign(c)

    for c in range(ncore):
        slot = 0
        for (b, p0) in tiles:
            for j in range(128):
                node = order[c, p0 + j]
                if node < 0:
                    slot += b  # dummy: idx 0, unmasked (finite junk)
                    continue
                d = deg[node]
                rows = (grow[ssrc[starts[node]:starts[node] + d]] - gbase)
                rows = np.sort(rows)  # negatives first, non-negatives last
                idx16[c, slot:slot + d] = rows.astype(np.int16)
                for s in range(slot + d, slot + b):
                    mask[c, s % 128, s // 128] = MASKVAL
                slot += b
        assert slot == totc * 128
        # verify per-tile trailing idx
        soff = 0
        for (b, p0) in tiles:
            assert idx16[c, soff + b * 128 - 1] >= 0
            soff += b * 128

    # wrap idx into the [16, n/16] layout, replicated across 128 partitions
    idxw = np.zeros((ncore, 128, totc * 8), np.int16)
    for c in range(ncore):
        w = idx16[c].reshape(totc * 8, 16).T  # idx i -> [i%16, i//16]
        idxw[c] = np.tile(w, (8, 1))

    return dict(deg=deg, bucket=bucket, ncap_b=ncap_b, ncap=ncap, ng=ng,
                gbase=gbase, tiles=tiles, totc=totc, nt=len(tiles),
                order=order, grow=grow, idxw=idxw, mask=mask, colmap=colmap)


def _patterns():
    """Agg one-hot patterns S_{b,k} [128, 32] f16, concatenated; plus offsets."""
    pats, offs = [], {}
    col = 0
    for b in BUCKETS:
        kd = (32 * b) // 128  # chunks per 32-node block
        offs[b] = col
        for k in range(kd):
            p = np.zeros((128, 32), np.float16)
            for q in range(128):
                p[q, (k * 128 + q) // b] = 1.0
            pats.append(p)
            col += 32
    return np.concatenate(pats, axis=1), offs


def _vmats():
    """V_b [128,128] f16 per bucket: hT col q = h row pperm_b[q], so the
    layer-2 table adopts the same qperm row order as layer 1's."""
    mats = []
    for b in BUCKETS:
        m = np.zeros((128, 128), np.float16)
        m[_pperm(b), np.arange(128)] = 1.0
        mats.append(m)
    return np.concatenate(mats, axis=1)


def _repmats():
    """E_b lhsT [128,128] f16 per bucket: out[p] = xs[p0(node(p))]; chunk-k
    invariant for pow2 buckets (no run straddle)."""
    mats, offs = [], {}
    col = 0
    for b in BUCKETS:
        m = np.zeros((128, 128), np.float16)
        for p in range(128):
            m[(p // b) * b, p] = 1.0  # lhsT[q, p]
        mats.append(m)
        offs[b] = col
        col += 128
    return np.concatenate(mats, axis=1), offs


# ---------------------------------------------------------------- weights
def prep_weights(W1_l, W1_r, b1_l, b1_r, a1, bias1, W2_l, W2_r, b2_l, b2_r,
                 a2, bias2):
    """Sign-permute features, fold a into tables; build packed weight mats."""
    p1 = np.argsort(a1 < 0, kind="stable")     # a1>=0 first
    n1p = int((a1 >= 0).sum())
    a1p = a1[p1]
    W1_lp, W1_rp = W1_l[:, p1], W1_r[:, p1]
    b1_lp, b1_rp = b1_l[p1], b1_r[p1]
    bias1p = bias1[p1]
    p2 = np.argsort(a2 < 0, kind="stable")
    n2p = int((a2 >= 0).sum())
    a2p = a2[p2]
    # W2 rows live in h-space -> permute rows by p1; columns by p2
    W2_lp, W2_rp = W2_l[p1][:, p2], W2_r[p1][:, p2]
    b2_lp, b2_rp = b2_l[p2], b2_r[p2]
    bias2p = bias2[p2]

    w1pack = np.concatenate([
        W1_lp * a1p[None, :], 1.5 * (W1_lp @ a1p)[:, None],
        W1_rp * a1p[None, :], 1.5 * (W1_rp @ a1p)[:, None]], axis=1)  # [128,130]
    b1pack = np.concatenate([
        b1_lp * a1p, [1.5 * (b1_lp @ a1p)],
        b1_rp * a1p, [1.5 * (b1_rp @ a1p)]])                          # [130]
    w2pack = np.concatenate([
        W2_lp * a2p[None, :], 1.5 * (W2_lp @ a2p)[:, None],
        W2_rp * a2p[None, :], 1.5 * (W2_rp @ a2p)[:, None]], axis=1)  # [64,34]
    b2pack = np.concatenate([
        b2_lp * a2p, [1.5 * (b2_lp @ a2p)],
        b2_rp * a2p, [1.5 * (b2_rp @ a2p)]])                          # [34]
    inv1 = (1.0 / a1p).astype(np.float32)
    inv2 = (1.0 / a2p).astype(np.float32)
    return dict(p1=p1, p2=p2, n1p=n1p, n2p=n2p, w1pack=w1pack, b1pack=b1pack,
                w2pack=w2pack, b2pack=b2pack, inv1=inv1, inv2=inv2,
                bias1p=bias1p.astype(np.float32), bias2p=bias2p.astype(np.float32))


# ---------------------------------------------------------------- device
def build_program(plan, wp, ncore):
    ncap, nt, totc, gbase = plan["ncap"], plan["nt"], plan["totc"], plan["gbase"]
    tiles = plan["tiles"]
    ng = plan["ng"]
    patv, patoffs = _patterns()
    edv, edoffs = _repmats()
    npat = patv.shape[1]

    nc = bacc.Bacc("TRN2", num_swdge_queues=4)
    XT = nc.declare_dram_parameter("XT", [128, ncap], F16, isOutput=False)
    W1P = nc.declare_dram_parameter("W1P", [128, 130], F16, isOutput=False)
    B1P = nc.declare_dram_parameter("B1P", [128, 130], F16, isOutput=False)
    W2P = nc.declare_dram_parameter("W2P", [64, 34], F16, isOutput=False)
    B2P = nc.declare_dram_parameter("B2P", [128, 34], F16, isOutput=False)
    IDX = nc.declare_dram_parameter("IDX", [128, totc * 8], mybir.dt.int16, isOutput=False)
    MASK = nc.declare_dram_parameter("MASK", [128, totc], F16, isOutput=False)
    PATS = nc.declare_dram_parameter("PATS", [128, npat], F16, isOutput=False)
    EDS = nc.declare_dram_parameter("EDS", [128, len(BUCKETS) * 128], F16, isOutput=False)
    VPERM = nc.declare_dram_parameter("VPERM", [128, len(BUCKETS) * 128], F16, isOutput=False)
    EYE = nc.declare_dram_parameter("EYE", [128, 128], F16, isOutput=False)
    INV1 = nc.declare_dram_parameter("INV1", [128, 64], F32, isOutput=False)
    BS1 = nc.declare_dram_parameter("BS1", [128, 64], F32, isOutput=False)
    INV2 = nc.declare_dram_parameter("INV2", [128, 16], F32, isOutput=False)
    BS2 = nc.declare_dram_parameter("BS2", [128, 16], F32, isOutput=False)
    OUT = nc.declare_dram_parameter("OUT", [ncap, 16], F32, isOutput=True)

    T1s = nc.dram_tensor("T1s", [ncap, 128], F16)
    shared = "Shared" if ncore > 4 else "Local"
    T1f = nc.dram_tensor("T1f", [ng, 128], F16, addr_space=shared)
    T2s = nc.dram_tensor("T2s", [ncap, 128], F16)
    T2f = nc.dram_tensor("T2f", [ng, 128], F16, addr_space=shared)
    # Local copies of the gathered tables: SWDGE random reads from the
    # Shared address space run ~3x slower than from Local DRAM, so copy
    # once (sequential, cheap) and gather from the Local replica.
    T1fl = nc.dram_tensor("T1fl", [ng, 128], F16)
    T2fl = nc.dram_tensor("T2fl", [ng, 128], F16)

    with TileContext(nc) as tc:
        nc.gpsimd.load_library(mlp)
        with tc.tile_pool(name="const", bufs=1) as cpool, \
             tc.tile_pool(name="work", bufs=5) as pool, \
             tc.tile_pool(name="zpool", bufs=6) as zpool, \
             tc.tile_pool(name="pz", bufs=3, space="PSUM") as pzpool, \
             tc.tile_pool(name="pa", bufs=2, space="PSUM") as papool:

            # persistent constants
            w1p = cpool.tile([128, 130], F16); nc.sync.dma_start(w1p[:], W1P[:])
            b1p = cpool.tile([128, 130], F16); nc.sync.dma_start(b1p[:], B1P[:])
            w2p = cpool.tile([64, 34], F16);   nc.sync.dma_start(w2p[:], W2P[:])
            b2p = cpool.tile([128, 34], F16);  nc.sync.dma_start(b2p[:], B2P[:])
            idxs = cpool.tile([128, totc * 8], mybir.dt.int16)
            nc.sync.dma_start(idxs[:], IDX[:])
            maskt = cpool.tile([128, totc], F16); nc.sync.dma_start(maskt[:], MASK[:])
            pats = cpool.tile([128, npat], F16); nc.sync.dma_start(pats[:], PATS[:])
            eds = cpool.tile([128, len(BUCKETS) * 128], F16)
            nc.sync.dma_start(eds[:], EDS[:])
            vperm = cpool.tile([128, len(BUCKETS) * 128], F16)
            nc.sync.dma_start(vperm[:], VPERM[:])
            eye = cpool.tile([128, 128], F16); nc.sync.dma_start(eye[:], EYE[:])
            inv1 = cpool.tile([128, 64], F32); nc.sync.dma_start(inv1[:], INV1[:])
            bs1 = cpool.tile([128, 64], F32); nc.sync.dma_start(bs1[:], BS1[:])
            inv2 = cpool.tile([128, 16], F32); nc.sync.dma_start(inv2[:], INV2[:])
            bs2 = cpool.tile([128, 16], F32); nc.sync.dma_start(bs2[:], BS2[:])
            xrt = cpool.tile([128, nt * 65], F16)      # x_r''' per sorted node
            h2rt = cpool.tile([128, nt * 17], F16)     # layer-2 r-side per node
            xs2 = [cpool.tile([128, 32 * 65], F16, name=f"xs{i}")
                   for i in range(4)]
            stg2 = [cpool.tile([128, 128], F16, name=f"stg{i}")
                    for i in range(3)]
            for t_ in xs2 + stg2:
                nc.gpsimd.memset(t_[:], 0.0)
            nc.gpsimd.memset(xrt[:], 0.0)
            nc.gpsimd.memset(h2rt[:], 0.0)

            # ---------------- phase A: layer-1 tables ----------------
            xrt_w, h2rt_w = [], []
            ni_regs = {cc: nc.gpsimd.to_reg(cc * 128)
                       for cc in sorted({min(b, 32) for b in BUCKETS})}
            for t in range(nt):
                xtc = pool.tile([128, 128], F16, tag="xtc")
                nc.sync.dma_start(xtc[:], XT[:, t * 128:(t + 1) * 128])
                psA = papool.tile([128, 130], F32, tag="tmp")
                stg = stg2[t % 3]
                nc.tensor.matmul(psA[:], xtc[:], w1p[:], start=True, stop=True)
                nc.vector.tensor_add(stg[:, 0:65], psA[:, 0:65], b1p[:, 0:65])
                nc.vector.tensor_scalar(stg[:, 65:66], psA[:, 64:65], 0.0, 1.0,
                                        mybir.AluOpType.mult, mybir.AluOpType.add)
                xrt_w.append(nc.vector.tensor_add(
                    xrt[:, t * 65:(t + 1) * 65],
                    psA[:, 65:130], b1p[:, 65:130]).ins)
                nc.sync.dma_start(T1s[t * 128:(t + 1) * 128, :], stg[:])
            if not os.environ.get("GAT_SKIP_CC"):
                nc.gpsimd.collective_compute(
                    "AllGather", mybir.AluOpType.bypass,
                    replica_groups=[list(range(ncore))],
                    ins=[T1s[:]], outs=[T1f[:]])
            else:
                nc.sync.dma_start(T1f[0:ncap, :], T1s[:, :])
            hg = ng // 2
            nc.sync.dma_start(T1fl[0:hg, :], T1f[0:hg, :])
            nc.scalar.dma_start(T1fl[hg:, :], T1f[hg:, :])

            # ---------------- phase C/E: per-layer edge phases ----------------
            qrr = [0]  # SWDGE queue round-robin

            def layer(F, Tf, xr_src, xr_w, n_pos, emit, xr_dep=None):
                K = 3      # pass2 (softmax+agg) lags pass1 by K subs

                def pass2(w):
                    C, b, prow = w["C"], w["b"], w["prow"]
                    az, pm, zt, psa = w["az"], w["pm"], w["zt"], w["psa"]
                    rp = pool.tile([128, 32], F32, tag="rp")
                    rm = pool.tile([128, 32], F32, tag="rm")
                    nc.vector.reduce_sum(rp[:, 0:C], az[:, 0:C, 0:n_pos],
                                         axis=AX.X)
                    nc.vector.reduce_sum(rm[:, 0:C], az[:, 0:C, n_pos:F],
                                         axis=AX.X)
                    u = pool.tile([128, 32], F32, tag="u")
                    nc.vector.tensor_sub(u[:, 0:C], rp[:, 0:C], rm[:, 0:C])
                    nc.vector.tensor_add(u[:, 0:C], u[:, 0:C], pm[:, 0:C])
                    ex = pool.tile([128, 32], F16, tag="ex")
                    nc.scalar.activation(ex[:, 0:C], u[:, 0:C], AF.Exp,
                                         scale=0.4)
                    sv = pool.tile([128, 32 * 32], F16, tag="sv")
                    kd = (32 * b) // 128      # chunks per 32-node block
                    nblk = C // kd
                    pf = pats[:, :]
                    pat_ap = bass.AP(pf.tensor, pf.offset + patoffs[b],
                                     [[pf.ap[0][0], 128], [0, nblk],
                                      [1, kd * 32]])
                    svv = sv.rearrange("p (n m) -> p n m", m=kd * 32)[:, 0:nblk, :]
                    exv = ex.rearrange("p (n k) -> p n k", k=kd)[:, 0:nblk, :]
                    exb = exv.unsqueeze(3).broadcast_to([128, nblk, kd, 32])
                    nc.vector.tensor_mul(
                        svv.rearrange("p n (k m) -> p n k m", m=32), pat_ap, exb)
                    for c in range(C):
                        blk = prow // 32 + c // kd
                        nc.tensor.matmul(
                            psa[32 * blk:32 * blk + 32, :],
                            sv[:, c * 32:(c + 1) * 32],
                            zt[:, c, 0:F + 2],
                            start=(c % kd == 0), stop=(c % kd == kd - 1),
                            tile_position=(0, 32 * blk), skip_group_check=True)
                    if w["last"]:
                        emit(w["ti"], psa)

                pend = []
                soff = 0   # chunk offset
                for ti, (b, p0) in enumerate(tiles):
                    xs = xs2[ti % 4]
                    subs = [(0, b)] if b <= 32 else [(0, 32), (64, 32)]
                    psa = papool.tile([128, F + 2], F32, tag="psa", bufs=3)
                    for si, (prow, C) in enumerate(subs):
                        zt = zpool.tile([128, 32, 128], F16, tag="zt")
                        if os.environ.get("GAT_SKIP_GATHER"):
                            nc.sync.dma_start(zt[:, 0:C, :],
                                              Tf[0:128, :].unsqueeze(1).broadcast_to([128, C, 128]))
                        else:
                            GMAX = 8  # chunks per gather (<=1024 idxs, ucode limit)
                            for g0 in range(0, C, GMAX):
                                g1 = min(g0 + GMAX, C)
                                nig = (g1 - g0) * 128
                                nc.gpsimd.dma_gather(
                                    zt[:, g0:g1, :], Tf[gbase:, :],
                                    idxs[:, (soff + g0) * 8:(soff + g1) * 8],
                                    nig, nig, 128, queue_num=qrr[0] % 4)
                                qrr[0] += 1
                        # spread xr rows: node j of chunk c at partition j*b.
                        # For b>=16 the table is qperm-ordered so slot-j's
                        # nodes sit at contiguous rows [j*C, (j+1)*C).
                        npchunk = 128 // b
                        xsf = xs[:, :]
                        sps = xsf.ap[0][0]
                        if b >= 16:
                            for j in range(npchunk):
                                src = xr_src[prow + j * C:prow + (j + 1) * C,
                                             ti * (F + 1):(ti + 1) * (F + 1)]
                                dst = xs[j * b:j * b + 1, 0:C * (F + 1)]
                                eng = nc.sync if j % 2 == 0 else nc.scalar
                                iv = eng.dma_start(dst, src)
                                if xr_dep is not None:
                                    add_dep_helper(iv.ins, xr_dep[ti], sync=True,
                                                   reason="spread reads xr table")
                        else:
                            for c in range(C):
                                src = xr_src[prow + c * npchunk:prow + (c + 1) * npchunk,
                                             ti * (F + 1):(ti + 1) * (F + 1)]
                                dst = bass.AP(xsf.tensor, xsf.offset + c * (F + 1),
                                              [[sps * b, npchunk], [1, F + 1]])
                                eng = nc.sync if c % 2 == 0 else nc.scalar
                                iv = eng.dma_start(dst, src)
                                if xr_dep is not None:
                                    add_dep_helper(iv.ins, xr_dep[ti], sync=True,
                                                   reason="spread reads xr table")
                        # z' psum: vals + q separately (bank-aligned)
                        pzq = papool.tile([128, 32], F32, tag="tmp")
                        cpg = 512 // F
                        xsv = xs[:, 0:C * (F + 1)].rearrange("p (c f) -> p c f", f=F + 1)
                        az = pool.tile([128, 32, F], F16, tag="az")
                        for c0 in range(0, C, cpg):
                            c1 = min(c0 + cpg, C)
                            pz = pzpool.tile([128, cpg * F], F32, tag="pz")
                            nc.tensor.matmul(pz[:, 0:(c1 - c0) * F], eye[:],
                                             zt[:, c0:c1, 0:F],
                                             start=True, stop=False)
                            nc.tensor.matmul(pz[:, 0:(c1 - c0) * F],
                                             eds[:, edoffs[b]:edoffs[b] + 128],
                                             xsv[:, c0:c1, 0:F],
                                             start=False, stop=True)
                            pzv = pz.rearrange("p (c f) -> p c f", f=F)[:, 0:c1 - c0, :]
                            nc.scalar.activation(az[:, c0:c1, :], pzv[:, :, :], AF.Abs)
                        ztf = zt[:, :, :]
                        zqcol = bass.AP(ztf.tensor, ztf.offset + F,
                                        [[ztf.ap[0][0], 128], [128, C]])
                        nc.tensor.matmul(pzq[:, 0:C], eye[:], zqcol,
                                         start=True, stop=False)
                        xqcol = bass.AP(xsf.tensor, xsf.offset + F,
                                        [[sps, 128], [F + 1, C]])
                        nc.tensor.matmul(pzq[:, 0:C],
                                         eds[:, edoffs[b]:edoffs[b] + 128],
                                         xqcol, start=False, stop=True)
                        pm = pool.tile([128, 32], F32, tag="pm")
                        nc.vector.tensor_add(pm[:, 0:C], pzq[:, 0:C],
                                             maskt[:, soff:soff + C])
                        pend.append(dict(C=C, b=b, prow=prow, az=az, pm=pm,
                                         zt=zt, psa=psa, ti=ti,
                                         last=(si == len(subs) - 1)))
                        soff += C
                        if len(pend) > K:
                            pass2(pend.pop(0))
                for w in pend:
                    pass2(w)

            # layer-1 epilogue: h, transpose, layer-2 tables
            def emit1(ti, psa):
                stg = stg2[ti % 3]
                rden = pool.tile([128, 1], F32, tag="rden")
                nc.vector.reciprocal(rden[:], psa[:, 65:66])
                h1 = pool.tile([128, 64], F32, tag="h1")
                nc.vector.tensor_scalar_mul(h1[:], psa[:, 0:64], rden[:])
                nc.vector.tensor_mul(h1[:], h1[:], inv1[:])
                nc.vector.tensor_add(h1[:], h1[:], bs1[:])
                h = pool.tile([128, 64], F16, tag="h")
                nc.scalar.activation(h[:], h1[:], AF.Relu)
                ptp = papool.tile([64, 128], F16, tag="tmp")
                bt = tiles[ti][0]
                nc.tensor.transpose(ptp[:], h[:],
                                    vperm[:, edoffs[bt]:edoffs[bt] + 128])
                hT = pool.tile([64, 128], F16, tag="hT")
                nc.scalar.copy(hT[:], ptp[:])
                ps2 = papool.tile([128, 34], F32, tag="tmp")
                nc.tensor.matmul(ps2[:], hT[:], w2p[:], start=True, stop=True)
                nc.vector.tensor_add(stg[:, 0:17], ps2[:, 0:17], b2p[:, 0:17])
                nc.vector.tensor_scalar(stg[:, 17:18], ps2[:, 16:17], 0.0, 1.0,
                                        mybir.AluOpType.mult, mybir.AluOpType.add)
                h2rt_w.append(nc.vector.tensor_add(
                    h2rt[:, ti * 17:(ti + 1) * 17],
                    ps2[:, 17:34], b2p[:, 17:34]).ins)
                nc.sync.dma_start(T2s[ti * 128:(ti + 1) * 128, :], stg[:, :])

            def emit2(ti, psa):
                rden = pool.tile([128, 1], F32, tag="rden")
                nc.vector.reciprocal(rden[:], psa[:, 17:18])
                o1 = pool.tile([128, 16], F32, tag="o1")
                nc.vector.tensor_scalar_mul(o1[:], psa[:, 0:16], rden[:])
                nc.vector.tensor_mul(o1[:], o1[:], inv2[:])
                nc.vector.tensor_add(o1[:], o1[:], bs2[:])
                nc.sync.dma_start(OUT[ti * 128:(ti + 1) * 128, :], o1[:])

            layer(64, T1fl, xrt, 65, wp["n1p"], emit1, xr_dep=xrt_w)
            if not os.environ.get("GAT_SKIP_CC"):
                nc.gpsimd.collective_compute(
                    "AllGather", mybir.AluOpType.bypass,
                    replica_groups=[list(range(ncore))],
                    ins=[T2s[:]], outs=[T2f[:]])
            else:
                nc.sync.dma_start(T2f[0:ncap, :], T2s[:, :])
            nc.sync.dma_start(T2fl[0:hg, :], T2f[0:hg, :])
            nc.scalar.dma_start(T2fl[hg:, :], T2f[hg:, :])
            layer(16, T2fl, h2rt, 17, wp["n2p"], emit2, xr_dep=h2rt_w)

    nc.compile()
    return nc


# ---------------------------------------------------------------- host entry
def kernel(x, edge_index, W1_l, W1_r, b1_l, b1_r, a1, bias1,
           W2_l, W2_r, b2_l, b2_r, a2, bias2, _run=None, _ncore=NCORE):
    x = np.asarray(x, np.float32)
    ei = np.asarray(edge_index)
    n = x.shape[0]
    loop = np.arange(n, dtype=ei.dtype)
    src = np.concatenate([np.asarray(ei[0]), loop]).astype(np.int64)
    dst = np.concatenate([np.asarray(ei[1]), loop]).astype(np.int64)

    plan = build_plan(src, dst, n, _ncore)
    wp = prep_weights(*[np.asarray(a, np.float32) for a in
                        (W1_l, W1_r, b1_l, b1_r, a1, bias1,
                         W2_l, W2_r, b2_l, b2_r, a2, bias2)])
    nc = build_program(plan, wp, _ncore)

    patv, _ = _patterns()
    edvv, _ = _repmats()
    vmv = _vmats()
    in_maps = []
    for c in range(_ncore):
        xt = np.zeros((128, plan["ncap"]), np.float16)
        ordc = plan["order"][c]
        valid = ordc >= 0
        xt[:, plan["colmap"][np.where(valid)[0]]] = \
            x[ordc[valid]].T.astype(np.float16)
        in_maps.append({
            "XT": xt,
            "W1P": wp["w1pack"].astype(np.float16),
            "B1P": np.tile(wp["b1pack"][None, :].astype(np.float16), (128, 1)),
            "W2P": wp["w2pack"].astype(np.float16),
            "B2P": np.tile(wp["b2pack"][None, :].astype(np.float16), (128, 1)),
            "IDX": plan["idxw"][c],
            "MASK": plan["mask"][c],
            "PATS": patv,
            "EDS": edvv,
            "VPERM": vmv,
            "EYE": np.eye(128, dtype=np.float16),
            "INV1": np.tile(wp["inv1"][None, :], (128, 1)),
            "BS1": np.tile(wp["bias1p"][None, :], (128, 1)),
            "INV2": np.tile(wp["inv2"][None, :], (128, 1)),
            "BS2": np.tile(wp["bias2p"][None, :], (128, 1)),
        })

    if _run is None:
        import time as _time
        res = run_bass_kernel_spmd(nc, in_maps, list(range(_ncore)))
        outs = [r["OUT"] for r in res.results]
        if os.environ.get("GAT_TRACE"):
            ts = []
            for _ in range(3):
                t0 = _time.time()
                run_bass_kernel_spmd(nc, in_maps, list(range(_ncore)))
                ts.append(_time.time() - t0)
            # min wall of a cached re-dispatch (includes host<->device I/O)
            print(f"HW exec time: {int(min(ts) * 1e9)} ns (e2e dispatch wall, "
                  f"runs: {[f'{t:.3f}s' for t in ts]})")
    else:
        outs = _run(nc, in_maps)   # test hook: returns list of OUT per core

    # unshard: rows sorted-order per core -> natural; cols: undo p2
    out = np.zeros((n, 16), np.float32)
    for c in range(_ncore):
        ordc = plan["order"][c]
        valid = ordc >= 0
        out[ordc[valid]] = outs[c][np.where(valid)[0]]
    inv_p2 = np.argsort(wp["p2"])
    return out[:, inv_p2].astype(np.float32)



# revision 32
# speedup vs baseline: 1.0480x; 1.0390x over previous
"""GATv2 (2-layer) Trainium2 Bass kernel, 8-core SPMD.

Dst-sharded graph parallel. See design notes at bottom of file.
"""

import os

import numpy as np

import concourse.bacc as bacc
import concourse.bass as bass
import concourse.mybir as mybir
from concourse.bass_utils import run_bass_kernel_spmd
from concourse.library_config import mlp
from concourse.tile import TileContext, add_dep_helper

F16 = mybir.dt.float16
F32 = mybir.dt.float32
AF = mybir.ActivationFunctionType
AX = mybir.AxisListType

NCORE = 8
BUCKETS = (4, 8, 16, 32, 64)
MASKVAL = -20000.0


# ---------------------------------------------------------------- structure
def _qperm(b):
    """Tile-partition permutation: the xr/table row of the node at slot
    position p lives at table row qperm[p] (xrt partition qperm[p]).
    For b>=16 this makes each per-j spread DMA source contiguous:
    positions {c*npchunk + j | c} <-> rows [j*C, (j+1)*C).  Identity for
    b<=8 (per-chunk spread path)."""
    q = np.arange(128)
    if b <= 8:
        return q
    C = min(b, 32)
    npchunk = 128 // b
    M = C * npchunk          # positions per sub (128 for b<=32, 64 for b=64)
    out = np.empty(128, np.int64)
    for base in range(0, 128, M):
        pl = np.arange(M)
        out[base + pl] = base + (pl % npchunk) * C + pl // npchunk
    return out


def _pperm(b):
    """Inverse of _qperm: position held at table row q."""
    q = _qperm(b)
    inv = np.empty(128, np.int64)
    inv[q] = np.arange(128)
    return inv


def build_plan(src, dst, n_nodes, ncore):
    npc = n_nodes // ncore
    deg = np.bincount(dst, minlength=n_nodes)
    assert deg.min() >= 1 and deg.max() <= BUCKETS[-1], (deg.min(), deg.max())
    bucket = np.full(n_nodes, BUCKETS[0], np.int64)
    for b in BUCKETS[1:]:
        bucket[deg > b // 2] = b
    core_of = np.arange(n_nodes) // npc

    ncap_b = {}
    for b in BUCKETS:
        cnt = max(((bucket == b) & (core_of == c)).sum() for c in range(ncore))
        ncap_b[b] = ((cnt + 127) // 128) * 128
    ncap = sum(ncap_b.values())
    ng = ncore * ncap
    gbase = ng // 2
    assert ng <= 65534, ng

    # tiles: (bucket, node offset within core's sorted order)
    tiles = []
    pos = 0
    for b in BUCKETS:
        for t in range(ncap_b[b] // 128):
            tiles.append((b, pos + t * 128))
        pos += ncap_b[b]
    totc = sum(b for b, _ in tiles)

    # per-core node order (sorted by bucket), -1 = dummy
    order = np.full((ncore, ncap), -1, np.int64)
    grow = np.full(n_nodes, -1, np.int64)   # global table row of node
    colmap = np.empty(ncap, np.int64)       # position -> table row (local)
    for (b, p0) in tiles:
        colmap[p0:p0 + 128] = p0 + _qperm(b)

    def _grow_assign(c):
        nodes = order[c]
        ok = nodes >= 0
        grow[nodes[ok]] = c * ncap + colmap[np.where(ok)[0]]

    for c in range(ncore):
        pos = 0
        for b in BUCKETS:
            nodes = np.where((bucket == b) & (core_of == c))[0]
            order[c, pos:pos + len(nodes)] = nodes
            pos += ncap_b[b]
        _grow_assign(c)

    # CSR of incoming edges by dst
    es = np.argsort(dst, kind="stable")
    ssrc = src[es]
    starts = np.zeros(n_nodes + 1, np.int64)
    np.cumsum(deg, out=starts[1:])

    # idx + mask per core; ensure each tile's last gather idx >= 0
    idx16 = np.zeros((ncore, totc * 128), np.int16)
    mask = np.zeros((ncore, 128, totc), np.float16)
    for c in range(ncore):
        # first fix node order so each tile's LAST real node can end >= 0
        tile_node_lists = []
        for (b, p0) in tiles:
            tile_node_lists.append(list(order[c, p0:p0 + 128]))
        for tl, (b, p0) in zip(tile_node_lists, tiles):
            last = tl[-1]
            if last < 0:
                continue  # dummy last -> idx 0, fine
            rows = grow[ssrc[starts[last]:starts[last] + deg[last]]] - gbase
            if deg[last] < b or (rows >= 0).any():
                continue  # pad slot last, or reorderable
            # swap with a node that can end non-negative
            for j in range(127):
                n2 = tl[j]
                if n2 < 0:
                    tl[j], tl[-1] = tl[-1], tl[j]
                    break
                r2 = grow[ssrc[starts[n2]:starts[n2] + deg[n2]]] - gbase
                if deg[n2] < b or (r2 >= 0).any():
                    tl[j], tl[-1] = tl[-1], tl[j]
                    break
            else:
                raise AssertionError("tile unfixable for trailing-negative")
        # rewrite order/grow after swaps
        for tl, (b, p0) in zip(tile_node_lists, tiles):
            order[c, p0:p0 + 128] = tl
        _grow_assign(c)

    for c in range(ncore):
        slot = 0
        for (b, p0) in tiles:
            for j in range(128):
                node = order[c, p0 + j]
                if node < 0:
                    slot += b  # dummy: idx 0, unmasked (finite junk)
                    continue
                d = deg[node]
                rows = (grow[ssrc[starts[node]:starts[node] + d]] - gbase)
                rows = np.sort(rows)  # negatives first, non-negatives last
                idx16[c, slot:slot + d] = rows.astype(np.int16)
                for s in range(slot + d, slot + b):
                    mask[c, s % 128, s // 128] = MASKVAL
                slot += b
        assert slot == totc * 128
        # verify per-tile trailing idx
        soff = 0
        for (b, p0) in tiles:
            assert idx16[c, soff + b * 128 - 1] >= 0
            soff += b * 128

    # wrap idx into the [16, n/16] layout, replicated across 128 partitions
    idxw = np.zeros((ncore, 128, totc * 8), np.int16)
    for c in range(ncore):
        w = idx16[c].reshape(totc * 8, 16).T  # idx i -> [i%16, i//16]
        idxw[c] = np.tile(w, (8, 1))

    return dict(deg=deg, bucket=bucket, ncap_b=ncap_b, ncap=ncap, ng=ng,
                gbase=gbase, tiles=tiles, totc=totc, nt=len(tiles),
                order=order, grow=grow, idxw=idxw, mask=mask, colmap=colmap)


def _patterns():
    """Agg one-hot patterns S_{b,k} [128, 32] f16, concatenated; plus offsets."""
    pats, offs = [], {}
    col = 0
    for b in BUCKETS:
        kd = (32 * b) // 128  # chunks per 32-node block
        offs[b] = col
        for k in range(kd):
            p = np.zeros((128, 32), np.float16)
            for q in range(128):
                p[q, (k * 128 + q) // b] = 1.0
            pats.append(p)
            col += 32
    return np.concatenate(pats, axis=1), offs


def _vmats():
    """V_b [128,128] f16 per bucket: hT col q = h row pperm_b[q], so the
    layer-2 table adopts the same qperm row order as layer 1's."""
    mats = []
    for b in BUCKETS:
        m = np.zeros((128, 128), np.float16)
        m[_pperm(b), np.arange(128)] = 1.0
        mats.append(m)
    return np.concatenate(mats, axis=1)


def _repmats():
    """E_b lhsT [128,128] f16 per bucket: out[p] = xs[p0(node(p))]; chunk-k
    invariant for pow2 buckets (no run straddle)."""
    mats, offs = [], {}
    col = 0
    for b in BUCKETS:
        m = np.zeros((128, 128), np.float16)
        for p in range(128):
            m[(p // b) * b, p] = 1.0  # lhsT[q, p]
        mats.append(m)
        offs[b] = col
        col += 128
    return np.concatenate(mats, axis=1), offs


# ---------------------------------------------------------------- weights
def prep_weights(W1_l, W1_r, b1_l, b1_r, a1, bias1, W2_l, W2_r, b2_l, b2_r,
                 a2, bias2):
    """Sign-permute features, fold a into tables; build packed weight mats."""
    p1 = np.argsort(a1 < 0, kind="stable")     # a1>=0 first
    n1p = int((a1 >= 0).sum())
    a1p = a1[p1]
    W1_lp, W1_rp = W1_l[:, p1], W1_r[:, p1]
    b1_lp, b1_rp = b1_l[p1], b1_r[p1]
    bias1p = bias1[p1]
    p2 = np.argsort(a2 < 0, kind="stable")
    n2p = int((a2 >= 0).sum())
    a2p = a2[p2]
    # W2 rows live in h-space -> permute rows by p1; columns by p2
    W2_lp, W2_rp = W2_l[p1][:, p2], W2_r[p1][:, p2]
    b2_lp, b2_rp = b2_l[p2], b2_r[p2]
    bias2p = bias2[p2]

    w1pack = np.concatenate([
        W1_lp * a1p[None, :], 1.5 * (W1_lp @ a1p)[:, None],
        W1_rp * a1p[None, :], 1.5 * (W1_rp @ a1p)[:, None]], axis=1)  # [128,130]
    b1pack = np.concatenate([
        b1_lp * a1p, [1.5 * (b1_lp @ a1p)],
        b1_rp * a1p, [1.5 * (b1_rp @ a1p)]])                          # [130]
    w2pack = np.concatenate([
        W2_lp * a2p[None, :], 1.5 * (W2_lp @ a2p)[:, None],
        W2_rp * a2p[None, :], 1.5 * (W2_rp @ a2p)[:, None]], axis=1)  # [64,34]
    b2pack = np.concatenate([
        b2_lp * a2p, [1.5 * (b2_lp @ a2p)],
        b2_rp * a2p, [1.5 * (b2_rp @ a2p)]])                          # [34]
    inv1 = (1.0 / a1p).astype(np.float32)
    inv2 = (1.0 / a2p).astype(np.float32)
    return dict(p1=p1, p2=p2, n1p=n1p, n2p=n2p, w1pack=w1pack, b1pack=b1pack,
                w2pack=w2pack, b2pack=b2pack, inv1=inv1, inv2=inv2,
                bias1p=bias1p.astype(np.float32), bias2p=bias2p.astype(np.float32))


# ---------------------------------------------------------------- device
def build_program(plan, wp, ncore):
    ncap, nt, totc, gbase = plan["ncap"], plan["nt"], plan["totc"], plan["gbase"]
    tiles = plan["tiles"]
    ng = plan["ng"]
    patv, patoffs = _patterns()
    edv, edoffs = _repmats()
    npat = patv.shape[1]

    nc = bacc.Bacc("TRN2", num_swdge_queues=4)
    XT = nc.declare_dram_parameter("XT", [128, ncap], F16, isOutput=False)
    W1P = nc.declare_dram_parameter("W1P", [128, 130], F16, isOutput=False)
    B1P = nc.declare_dram_parameter("B1P", [128, 130], F16, isOutput=False)
    W2P = nc.declare_dram_parameter("W2P", [64, 34], F16, isOutput=False)
    B2P = nc.declare_dram_parameter("B2P", [128, 34], F16, isOutput=False)
    IDX = nc.declare_dram_parameter("IDX", [128, totc * 8], mybir.dt.int16, isOutput=False)
    MASK = nc.declare_dram_parameter("MASK", [128, totc], F16, isOutput=False)
    PATS = nc.declare_dram_parameter("PATS", [128, npat], F16, isOutput=False)
    EDS = nc.declare_dram_parameter("EDS", [128, len(BUCKETS) * 128], F16, isOutput=False)
    VPERM = nc.declare_dram_parameter("VPERM", [128, len(BUCKETS) * 128], F16, isOutput=False)
    EYE = nc.declare_dram_parameter("EYE", [128, 128], F16, isOutput=False)
    INV1 = nc.declare_dram_parameter("INV1", [128, 64], F32, isOutput=False)
    BS1 = nc.declare_dram_parameter("BS1", [128, 64], F32, isOutput=False)
    INV2 = nc.declare_dram_parameter("INV2", [128, 16], F32, isOutput=False)
    BS2 = nc.declare_dram_parameter("BS2", [128, 16], F32, isOutput=False)
    OUT = nc.declare_dram_parameter("OUT", [ncap, 16], F32, isOutput=True)

    T1s = nc.dram_tensor("T1s", [ncap, 128], F16)
    shared = "Shared" if ncore > 4 else "Local"
    T1f = nc.dram_tensor("T1f", [ng, 128], F16, addr_space=shared)
    T2s = nc.dram_tensor("T2s", [ncap, 128], F16)
    T2f = nc.dram_tensor("T2f", [ng, 128], F16, addr_space=shared)
    # Local copies of the gathered tables: SWDGE random reads from the
    # Shared address space run ~3x slower than from Local DRAM, so copy
    # once (sequential, cheap) and gather from the Local replica.
    T1fl = nc.dram_tensor("T1fl", [ng, 128], F16)
    T2fl = nc.dram_tensor("T2fl", [ng, 128], F16)

    with TileContext(nc) as tc:
        nc.gpsimd.load_library(mlp)
        with tc.tile_pool(name="const", bufs=1) as cpool, \
             tc.tile_pool(name="work", bufs=6) as pool, \
             tc.tile_pool(name="zpool", bufs=7) as zpool, \
             tc.tile_pool(name="pz", bufs=3, space="PSUM") as pzpool, \
             tc.tile_pool(name="pa", bufs=2, space="PSUM") as papool:

            # persistent constants
            w1p = cpool.tile([128, 130], F16); nc.sync.dma_start(w1p[:], W1P[:])
            b1p = cpool.tile([128, 130], F16); nc.sync.dma_start(b1p[:], B1P[:])
            w2p = cpool.tile([64, 34], F16);   nc.sync.dma_start(w2p[:], W2P[:])
            b2p = cpool.tile([128, 34], F16);  nc.sync.dma_start(b2p[:], B2P[:])
            idxs = cpool.tile([128, totc * 8], mybir.dt.int16)
            nc.sync.dma_start(idxs[:], IDX[:])
            maskt = cpool.tile([128, totc], F16); nc.sync.dma_start(maskt[:], MASK[:])
            pats = cpool.tile([128, npat], F16); nc.sync.dma_start(pats[:], PATS[:])
            eds = cpool.tile([128, len(BUCKETS) * 128], F16)
            nc.sync.dma_start(eds[:], EDS[:])
            vperm = cpool.tile([128, len(BUCKETS) * 128], F16)
            nc.sync.dma_start(vperm[:], VPERM[:])
            eye = cpool.tile([128, 128], F16); nc.sync.dma_start(eye[:], EYE[:])
            inv1 = cpool.tile([128, 64], F32); nc.sync.dma_start(inv1[:], INV1[:])
            bs1 = cpool.tile([128, 64], F32); nc.sync.dma_start(bs1[:], BS1[:])
            inv2 = cpool.tile([128, 16], F32); nc.sync.dma_start(inv2[:], INV2[:])
            bs2 = cpool.tile([128, 16], F32); nc.sync.dma_start(bs2[:], BS2[:])
            xrt = cpool.tile([128, nt * 65], F16)      # x_r''' per sorted node
            h2rt = cpool.tile([128, nt * 17], F16)     # layer-2 r-side per node
            xs2 = [cpool.tile([128, 32 * 65], F16, name=f"xs{i}")
                   for i in range(4)]
            stg2 = [cpool.tile([128, 128], F16, name=f"stg{i}")
                    for i in range(3)]
            for t_ in xs2 + stg2:
                nc.gpsimd.memset(t_[:], 0.0)
            nc.gpsimd.memset(xrt[:], 0.0)
            nc.gpsimd.memset(h2rt[:], 0.0)

            # ---------------- phase A: layer-1 tables ----------------
            xrt_w, h2rt_w = [], []
            ni_regs = {cc: nc.gpsimd.to_reg(cc * 128)
                       for cc in sorted({min(b, 32) for b in BUCKETS})}
            for t in range(nt):
                xtc = pool.tile([128, 128], F16, tag="xtc")
                nc.sync.dma_start(xtc[:], XT[:, t * 128:(t + 1) * 128])
                psA = papool.tile([128, 130], F32, tag="tmp")
                stg = stg2[t % 3]
                nc.tensor.matmul(psA[:], xtc[:], w1p[:], start=True, stop=True)
                nc.vector.tensor_add(stg[:, 0:65], psA[:, 0:65], b1p[:, 0:65])
                nc.vector.tensor_scalar(stg[:, 65:66], psA[:, 64:65], 0.0, 1.0,
                                        mybir.AluOpType.mult, mybir.AluOpType.add)
                xrt_w.append(nc.vector.tensor_add(
                    xrt[:, t * 65:(t + 1) * 65],
                    psA[:, 65:130], b1p[:, 65:130]).ins)
                nc.sync.dma_start(T1s[t * 128:(t + 1) * 128, :], stg[:])
            if not os.environ.get("GAT_SKIP_CC"):
                nc.gpsimd.collective_compute(
                    "AllGather", mybir.AluOpType.bypass,
                    replica_groups=[list(range(ncore))],
                    ins=[T1s[:]], outs=[T1f[:]])
            else:
                nc.sync.dma_start(T1f[0:ncap, :], T1s[:, :])
            hg = ng // 2
            nc.sync.dma_start(T1fl[0:hg, :], T1f[0:hg, :])
            nc.scalar.dma_start(T1fl[hg:, :], T1f[hg:, :])

            # ---------------- phase C/E: per-layer edge phases ----------------
            qrr = [0]  # SWDGE queue round-robin

            def layer(F, Tf, xr_src, xr_w, n_pos, emit, xr_dep=None):
                K = 4      # pass2 (softmax+agg) lags pass1 by K subs

                def pass2(w):
                    C, b, prow = w["C"], w["b"], w["prow"]
                    az, pm, zt, psa = w["az"], w["pm"], w["zt"], w["psa"]
                    rp = pool.tile([128, 32], F32, tag="rp")
                    rm = pool.tile([128, 32], F32, tag="rm")
                    nc.vector.reduce_sum(rp[:, 0:C], az[:, 0:C, 0:n_pos],
                                         axis=AX.X)
                    nc.vector.reduce_sum(rm[:, 0:C], az[:, 0:C, n_pos:F],
                                         axis=AX.X)
                    u = pool.tile([128, 32], F32, tag="u")
                    nc.vector.tensor_sub(u[:, 0:C], rp[:, 0:C], rm[:, 0:C])
                    nc.vector.tensor_add(u[:, 0:C], u[:, 0:C], pm[:, 0:C])
                    ex = pool.tile([128, 32], F16, tag="ex")
                    nc.scalar.activation(ex[:, 0:C], u[:, 0:C], AF.Exp,
                                         scale=0.4)
                    sv = pool.tile([128, 32 * 32], F16, tag="sv")
                    kd = (32 * b) // 128      # chunks per 32-node block
                    nblk = C // kd
                    pf = pats[:, :]
                    pat_ap = bass.AP(pf.tensor, pf.offset + patoffs[b],
                                     [[pf.ap[0][0], 128], [0, nblk],
                                      [1, kd * 32]])
                    svv = sv.rearrange("p (n m) -> p n m", m=kd * 32)[:, 0:nblk, :]
                    exv = ex.rearrange("p (n k) -> p n k", k=kd)[:, 0:nblk, :]
                    exb = exv.unsqueeze(3).broadcast_to([128, nblk, kd, 32])
                    nc.vector.tensor_mul(
                        svv.rearrange("p n (k m) -> p n k m", m=32), pat_ap, exb)
                    for c in range(C):
                        blk = prow // 32 + c // kd
                        nc.tensor.matmul(
                            psa[32 * blk:32 * blk + 32, :],
                            sv[:, c * 32:(c + 1) * 32],
                            zt[:, c, 0:F + 2],
                            start=(c % kd == 0), stop=(c % kd == kd - 1),
                            tile_position=(0, 32 * blk), skip_group_check=True)
                    if w["last"]:
                        emit(w["ti"], psa)

                pend = []
                soff = 0   # chunk offset
                for ti, (b, p0) in enumerate(tiles):
                    xs = xs2[ti % 4]
                    subs = [(0, b)] if b <= 32 else [(0, 32), (64, 32)]
                    psa = papool.tile([128, F + 2], F32, tag="psa", bufs=3)
                    for si, (prow, C) in enumerate(subs):
                        zt = zpool.tile([128, 32, 128], F16, tag="zt")
                        if os.environ.get("GAT_SKIP_GATHER"):
                            nc.sync.dma_start(zt[:, 0:C, :],
                                              Tf[0:128, :].unsqueeze(1).broadcast_to([128, C, 128]))
                        else:
                            GMAX = 8  # chunks per gather (<=1024 idxs, ucode limit)
                            for g0 in range(0, C, GMAX):
                                g1 = min(g0 + GMAX, C)
                                nig = (g1 - g0) * 128
                                nc.gpsimd.dma_gather(
                                    zt[:, g0:g1, :], Tf[gbase:, :],
                                    idxs[:, (soff + g0) * 8:(soff + g1) * 8],
                                    nig, nig, 128, queue_num=qrr[0] % 4)
                                qrr[0] += 1
                        # spread xr rows: node j of chunk c at partition j*b.
                        # For b>=16 the table is qperm-ordered so slot-j's
                        # nodes sit at contiguous rows [j*C, (j+1)*C).
                        npchunk = 128 // b
                        xsf = xs[:, :]
                        sps = xsf.ap[0][0]
                        if b >= 16:
                            for j in range(npchunk):
                                src = xr_src[prow + j * C:prow + (j + 1) * C,
                                             ti * (F + 1):(ti + 1) * (F + 1)]
                                dst = xs[j * b:j * b + 1, 0:C * (F + 1)]
                                eng = nc.sync if j % 2 == 0 else nc.scalar
                                iv = eng.dma_start(dst, src)
                                if xr_dep is not None:
                                    add_dep_helper(iv.ins, xr_dep[ti], sync=True,
                                                   reason="spread reads xr table")
                        else:
                            for c in range(C):
                                src = xr_src[prow + c * npchunk:prow + (c + 1) * npchunk,
                                             ti * (F + 1):(ti + 1) * (F + 1)]
                                dst = bass.AP(xsf.tensor, xsf.offset + c * (F + 1),
                                              [[sps * b, npchunk], [1, F + 1]])
                                eng = nc.sync if c % 2 == 0 else nc.scalar
                                iv = eng.dma_start(dst, src)
                                if xr_dep is not None:
                                    add_dep_helper(iv.ins, xr_dep[ti], sync=True,
                                                   reason="spread reads xr table")
                        # z' psum: vals + q separately (bank-aligned)
                        pzq = papool.tile([128, 32], F32, tag="tmp")
                        cpg = 512 // F
                        xsv = xs[:, 0:C * (F + 1)].rearrange("p (c f) -> p c f", f=F + 1)
                        az = pool.tile([128, 32, F], F16, tag="az")
                        for c0 in range(0, C, cpg):
                            c1 = min(c0 + cpg, C)
                            pz = pzpool.tile([128, cpg * F], F32, tag="pz")
                            nc.tensor.matmul(pz[:, 0:(c1 - c0) * F], eye[:],
                                             zt[:, c0:c1, 0:F],
                                             start=True, stop=False)
                            nc.tensor.matmul(pz[:, 0:(c1 - c0) * F],
                                             eds[:, edoffs[b]:edoffs[b] + 128],
                                             xsv[:, c0:c1, 0:F],
                                             start=False, stop=True)
                            pzv = pz.rearrange("p (c f) -> p c f", f=F)[:, 0:c1 - c0, :]
                            nc.scalar.activation(az[:, c0:c1, :], pzv[:, :, :], AF.Abs)
                        ztf = zt[:, :, :]
                        zqcol = bass.AP(ztf.tensor, ztf.offset + F,
                                        [[ztf.ap[0][0], 128], [128, C]])
                        nc.tensor.matmul(pzq[:, 0:C], eye[:], zqcol,
                                         start=True, stop=False)
                        xqcol = bass.AP(xsf.tensor, xsf.offset + F,
                                        [[sps, 128], [F + 1, C]])
                        nc.tensor.matmul(pzq[:, 0:C],
                                         eds[:, edoffs[b]:edoffs[b] + 128],
                                         xqcol, start=False, stop=True)
                        pm = pool.tile([128, 32], F32, tag="pm")
                        nc.vector.tensor_add(pm[:, 0:C], pzq[:, 0:C],
                                             maskt[:, soff:soff + C])
                        pend.append(dict(C=C, b=b, prow=prow, az=az, pm=pm,
                                         zt=zt, psa=psa, ti=ti,
                                         last=(si == len(subs) - 1)))
                        soff += C
                        if len(pend) > K:
                            pass2(pend.pop(0))
                for w in pend:
                    pass2(w)

            # layer-1 epilogue: h, transpose, layer-2 tables
            def emit1(ti, psa):
                stg = stg2[ti % 3]
                rden = pool.tile([128, 1], F32, tag="rden")
                nc.vector.reciprocal(rden[:], psa[:, 65:66])
                h1 = pool.tile([128, 64], F32, tag="h1")
                nc.vector.tensor_scalar_mul(h1[:], psa[:, 0:64], rden[:])
                nc.vector.tensor_mul(h1[:], h1[:], inv1[:])
                nc.vector.tensor_add(h1[:], h1[:], bs1[:])
                h = pool.tile([128, 64], F16, tag="h")
                nc.scalar.activation(h[:], h1[:], AF.Relu)
                ptp = papool.tile([64, 128], F16, tag="tmp")
                bt = tiles[ti][0]
                nc.tensor.transpose(ptp[:], h[:],
                                    vperm[:, edoffs[bt]:edoffs[bt] + 128])
                hT = pool.tile([64, 128], F16, tag="hT")
                nc.scalar.copy(hT[:], ptp[:])
                ps2 = papool.tile([128, 34], F32, tag="tmp")
                nc.tensor.matmul(ps2[:], hT[:], w2p[:], start=True, stop=True)
                nc.vector.tensor_add(stg[:, 0:17], ps2[:, 0:17], b2p[:, 0:17])
                nc.vector.tensor_scalar(stg[:, 17:18], ps2[:, 16:17], 0.0, 1.0,
                                        mybir.AluOpType.mult, mybir.AluOpType.add)
                h2rt_w.append(nc.vector.tensor_add(
                    h2rt[:, ti * 17:(ti + 1) * 17],
                    ps2[:, 17:34], b2p[:, 17:34]).ins)
                nc.sync.dma_start(T2s[ti * 128:(ti + 1) * 128, :], stg[:, :])

            def emit2(ti, psa):
                rden = pool.tile([128, 1], F32, tag="rden")
                nc.vector.reciprocal(rden[:], psa[:, 17:18])
                o1 = pool.tile([128, 16], F32, tag="o1")
                nc.vector.tensor_scalar_mul(o1[:], psa[:, 0:16], rden[:])
                nc.vector.tensor_mul(o1[:], o1[:], inv2[:])
                nc.vector.tensor_add(o1[:], o1[:], bs2[:])
                nc.sync.dma_start(OUT[ti * 128:(ti + 1) * 128, :], o1[:])

            layer(64, T1fl, xrt, 65, wp["n1p"], emit1, xr_dep=xrt_w)
            if not os.environ.get("GAT_SKIP_CC"):
                nc.gpsimd.collective_compute(
                    "AllGather", mybir.AluOpType.bypass,
                    replica_groups=[list(range(ncore))],
                    ins=[T2s[:]], outs=[T2f[:]])
            else:
                nc.sync.dma_start(T2f[0:ncap, :], T2s[:, :])
            nc.sync.dma_start(T2fl[0:hg, :], T2f[0:hg, :])
            nc.scalar.dma_start(T2fl[hg:, :], T2f[hg:, :])
            layer(16, T2fl, h2rt, 17, wp["n2p"], emit2, xr_dep=h2rt_w)

    nc.compile()
    return nc


# ---------------------------------------------------------------- host entry
def kernel(x, edge_index, W1_l, W1_r, b1_l, b1_r, a1, bias1,
           W2_l, W2_r, b2_l, b2_r, a2, bias2, _run=None, _ncore=NCORE):
    x = np.asarray(x, np.float32)
    ei = np.asarray(edge_index)
    n = x.shape[0]
    loop = np.arange(n, dtype=ei.dtype)
    src = np.concatenate([np.asarray(ei[0]), loop]).astype(np.int64)
    dst = np.concatenate([np.asarray(ei[1]), loop]).astype(np.int64)

    plan = build_plan(src, dst, n, _ncore)
    wp = prep_weights(*[np.asarray(a, np.float32) for a in
                        (W1_l, W1_r, b1_l, b1_r, a1, bias1,
                         W2_l, W2_r, b2_l, b2_r, a2, bias2)])
    nc = build_program(plan, wp, _ncore)

    patv, _ = _patterns()
    edvv, _ = _repmats()
    vmv = _vmats()
    in_maps = []
    for c in range(_ncore):
        xt = np.zeros((128, plan["ncap"]), np.float16)
        ordc = plan["order"][c]
        valid = ordc >= 0
        xt[:, plan["colmap"][np.where(valid)[0]]] = \
            x[ordc[valid]].T.astype(np.float16)
        in_maps.append({
            "XT": xt,
            "W1P": wp["w1pack"].astype(np.float16),
            "B1P": np.tile(wp["b1pack"][None, :].astype(np.float16), (128, 1)),
            "W2P": wp["w2pack"].astype(np.float16),
            "B2P": np.tile(wp["b2pack"][None, :].astype(np.float16), (128, 1)),
            "IDX": plan["idxw"][c],
            "MASK": plan["mask"][c],
            "PATS": patv,
            "EDS": edvv,
            "VPERM": vmv,
            "EYE": np.eye(128, dtype=np.float16),
            "INV1": np.tile(wp["inv1"][None, :], (128, 1)),
            "BS1": np.tile(wp["bias1p"][None, :], (128, 1)),
            "INV2": np.tile(wp["inv2"][None, :], (128, 1)),
            "BS2": np.tile(wp["bias2p"][None, :], (128, 1)),
        })

    if _run is None:
        import time as _time
        res = run_bass_kernel_spmd(nc, in_maps, list(range(_ncore)))
        outs = [r["OUT"] for r in res.results]
        if os.environ.get("GAT_TRACE"):
            ts = []
            for _ in range(3):
                t0 = _time.time()
                run_bass_kernel_spmd(nc, in_maps, list(range(_ncore)))
                ts.append(_time.time() - t0)
            # min wall of a cached re-dispatch (includes host<->device I/O)
            print(f"HW exec time: {int(min(ts) * 1e9)} ns (e2e dispatch wall, "
                  f"runs: {[f'{t:.3f}s' for t in ts]})")
    else:
        outs = _run(nc, in_maps)   # test hook: returns list of OUT per core

    # unshard: rows sorted-order per core -> natural; cols: undo p2
    out = np.zeros((n, 16), np.float32)
    for c in range(_ncore):
        ordc = plan["order"][c]
        valid = ordc >= 0
        out[ordc[valid]] = outs[c][np.where(valid)[0]]
    inv_p2 = np.argsort(wp["p2"])
    return out[:, inv_p2].astype(np.float32)



# revision 33
# speedup vs baseline: 1.1132x; 1.0622x over previous
"""GATv2 (2-layer) Trainium2 Bass kernel, 8-core SPMD.

Dst-sharded graph parallel. See design notes at bottom of file.
"""

import os

import numpy as np

import concourse.bacc as bacc
import concourse.bass as bass
import concourse.mybir as mybir
from concourse.bass_utils import run_bass_kernel_spmd
from concourse.library_config import mlp
from concourse.tile import TileContext, add_dep_helper

F16 = mybir.dt.float16
F32 = mybir.dt.float32
AF = mybir.ActivationFunctionType
AX = mybir.AxisListType

NCORE = 8
BUCKETS = (4, 8, 16, 32, 64)
MASKVAL = -20000.0


# ---------------------------------------------------------------- structure
def _qperm(b):
    """Tile-partition permutation: the xr/table row of the node at slot
    position p lives at table row qperm[p] (xrt partition qperm[p]).
    For b>=16 this makes each per-j spread DMA source contiguous:
    positions {c*npchunk + j | c} <-> rows [j*C, (j+1)*C).  Identity for
    b<=8 (per-chunk spread path)."""
    q = np.arange(128)
    if b <= 8:
        return q
    C = min(b, 32)
    npchunk = 128 // b
    M = C * npchunk          # positions per sub (128 for b<=32, 64 for b=64)
    out = np.empty(128, np.int64)
    for base in range(0, 128, M):
        pl = np.arange(M)
        out[base + pl] = base + (pl % npchunk) * C + pl // npchunk
    return out


def _pperm(b):
    """Inverse of _qperm: position held at table row q."""
    q = _qperm(b)
    inv = np.empty(128, np.int64)
    inv[q] = np.arange(128)
    return inv


def build_plan(src, dst, n_nodes, ncore):
    npc = n_nodes // ncore
    deg = np.bincount(dst, minlength=n_nodes)
    assert deg.min() >= 1 and deg.max() <= BUCKETS[-1], (deg.min(), deg.max())
    bucket = np.full(n_nodes, BUCKETS[0], np.int64)
    for b in BUCKETS[1:]:
        bucket[deg > b // 2] = b
    core_of = np.arange(n_nodes) // npc

    ncap_b = {}
    for b in BUCKETS:
        cnt = max(((bucket == b) & (core_of == c)).sum() for c in range(ncore))
        ncap_b[b] = ((cnt + 127) // 128) * 128
    ncap = sum(ncap_b.values())
    ng = ncore * ncap
    gbase = ng // 2
    assert ng <= 65534, ng

    # tiles: (bucket, node offset within core's sorted order)
    tiles = []
    pos = 0
    for b in BUCKETS:
        for t in range(ncap_b[b] // 128):
            tiles.append((b, pos + t * 128))
        pos += ncap_b[b]
    totc = sum(b for b, _ in tiles)

    # per-core node order (sorted by bucket), -1 = dummy
    order = np.full((ncore, ncap), -1, np.int64)
    grow = np.full(n_nodes, -1, np.int64)   # global table row of node
    colmap = np.empty(ncap, np.int64)       # position -> table row (local)
    for (b, p0) in tiles:
        colmap[p0:p0 + 128] = p0 + _qperm(b)

    def _grow_assign(c):
        nodes = order[c]
        ok = nodes >= 0
        grow[nodes[ok]] = c * ncap + colmap[np.where(ok)[0]]

    for c in range(ncore):
        pos = 0
        for b in BUCKETS:
            nodes = np.where((bucket == b) & (core_of == c))[0]
            order[c, pos:pos + len(nodes)] = nodes
            pos += ncap_b[b]
        _grow_assign(c)

    # CSR of incoming edges by dst
    es = np.argsort(dst, kind="stable")
    ssrc = src[es]
    starts = np.zeros(n_nodes + 1, np.int64)
    np.cumsum(deg, out=starts[1:])

    # idx + mask per core; ensure each tile's last gather idx >= 0
    idx16 = np.zeros((ncore, totc * 128), np.int16)
    mask = np.zeros((ncore, 128, totc), np.float16)
    for c in range(ncore):
        # first fix node order so each tile's LAST real node can end >= 0
        tile_node_lists = []
        for (b, p0) in tiles:
            tile_node_lists.append(list(order[c, p0:p0 + 128]))
        for tl, (b, p0) in zip(tile_node_lists, tiles):
            last = tl[-1]
            if last < 0:
                continue  # dummy last -> idx 0, fine
            rows = grow[ssrc[starts[last]:starts[last] + deg[last]]] - gbase
            if deg[last] < b or (rows >= 0).any():
                continue  # pad slot last, or reorderable
            # swap with a node that can end non-negative
            for j in range(127):
                n2 = tl[j]
                if n2 < 0:
                    tl[j], tl[-1] = tl[-1], tl[j]
                    break
                r2 = grow[ssrc[starts[n2]:starts[n2] + deg[n2]]] - gbase
                if deg[n2] < b or (r2 >= 0).any():
                    tl[j], tl[-1] = tl[-1], tl[j]
                    break
            else:
                raise AssertionError("tile unfixable for trailing-negative")
        # rewrite order/grow after swaps
        for tl, (b, p0) in zip(tile_node_lists, tiles):
            order[c, p0:p0 + 128] = tl
        _grow_assign(c)

    for c in range(ncore):
        slot = 0
        for (b, p0) in tiles:
            for j in range(128):
                node = order[c, p0 + j]
                if node < 0:
                    slot += b  # dummy: idx 0, unmasked (finite junk)
                    continue
                d = deg[node]
                rows = (grow[ssrc[starts[node]:starts[node] + d]] - gbase)
                rows = np.sort(rows)  # negatives first, non-negatives last
                idx16[c, slot:slot + d] = rows.astype(np.int16)
                for s in range(slot + d, slot + b):
                    mask[c, s % 128, s // 128] = MASKVAL
                slot += b
        assert slot == totc * 128
        # verify per-tile trailing idx
        soff = 0
        for (b, p0) in tiles:
            assert idx16[c, soff + b * 128 - 1] >= 0
            soff += b * 128

    # wrap idx into the [16, n/16] layout, replicated across 128 partitions
    idxw = np.zeros((ncore, 128, totc * 8), np.int16)
    for c in range(ncore):
        w = idx16[c].reshape(totc * 8, 16).T  # idx i -> [i%16, i//16]
        idxw[c] = np.tile(w, (8, 1))

    return dict(deg=deg, bucket=bucket, ncap_b=ncap_b, ncap=ncap, ng=ng,
                gbase=gbase, tiles=tiles, totc=totc, nt=len(tiles),
                order=order, grow=grow, idxw=idxw, mask=mask, colmap=colmap)


def _patterns():
    """Agg one-hot patterns S_{b,k} [128, 32] f16, concatenated; plus offsets."""
    pats, offs = [], {}
    col = 0
    for b in BUCKETS:
        kd = (32 * b) // 128  # chunks per 32-node block
        offs[b] = col
        for k in range(kd):
            p = np.zeros((128, 32), np.float16)
            for q in range(128):
                p[q, (k * 128 + q) // b] = 1.0
            pats.append(p)
            col += 32
    return np.concatenate(pats, axis=1), offs


def _vmats():
    """V_b [128,128] f16 per bucket: hT col q = h row pperm_b[q], so the
    layer-2 table adopts the same qperm row order as layer 1's."""
    mats = []
    for b in BUCKETS:
        m = np.zeros((128, 128), np.float16)
        m[_pperm(b), np.arange(128)] = 1.0
        mats.append(m)
    return np.concatenate(mats, axis=1)


def _repmats():
    """E_b lhsT [128,128] f16 per bucket: out[p] = xs[p0(node(p))]; chunk-k
    invariant for pow2 buckets (no run straddle)."""
    mats, offs = [], {}
    col = 0
    for b in BUCKETS:
        m = np.zeros((128, 128), np.float16)
        for p in range(128):
            m[(p // b) * b, p] = 1.0  # lhsT[q, p]
        mats.append(m)
        offs[b] = col
        col += 128
    return np.concatenate(mats, axis=1), offs


# ---------------------------------------------------------------- weights
def prep_weights(W1_l, W1_r, b1_l, b1_r, a1, bias1, W2_l, W2_r, b2_l, b2_r,
                 a2, bias2):
    """Sign-permute features, fold a into tables; build packed weight mats."""
    p1 = np.argsort(a1 < 0, kind="stable")     # a1>=0 first
    n1p = int((a1 >= 0).sum())
    a1p = a1[p1]
    W1_lp, W1_rp = W1_l[:, p1], W1_r[:, p1]
    b1_lp, b1_rp = b1_l[p1], b1_r[p1]
    bias1p = bias1[p1]
    p2 = np.argsort(a2 < 0, kind="stable")
    n2p = int((a2 >= 0).sum())
    a2p = a2[p2]
    # W2 rows live in h-space -> permute rows by p1; columns by p2
    W2_lp, W2_rp = W2_l[p1][:, p2], W2_r[p1][:, p2]
    b2_lp, b2_rp = b2_l[p2], b2_r[p2]
    bias2p = bias2[p2]

    w1pack = np.concatenate([
        W1_lp * a1p[None, :], 1.5 * (W1_lp @ a1p)[:, None],
        W1_rp * a1p[None, :], 1.5 * (W1_rp @ a1p)[:, None]], axis=1)  # [128,130]
    b1pack = np.concatenate([
        b1_lp * a1p, [1.5 * (b1_lp @ a1p)],
        b1_rp * a1p, [1.5 * (b1_rp @ a1p)]])                          # [130]
    w2pack = np.concatenate([
        W2_lp * a2p[None, :], 1.5 * (W2_lp @ a2p)[:, None],
        W2_rp * a2p[None, :], 1.5 * (W2_rp @ a2p)[:, None]], axis=1)  # [64,34]
    b2pack = np.concatenate([
        b2_lp * a2p, [1.5 * (b2_lp @ a2p)],
        b2_rp * a2p, [1.5 * (b2_rp @ a2p)]])                          # [34]
    inv1 = (1.0 / a1p).astype(np.float32)
    inv2 = (1.0 / a2p).astype(np.float32)
    return dict(p1=p1, p2=p2, n1p=n1p, n2p=n2p, w1pack=w1pack, b1pack=b1pack,
                w2pack=w2pack, b2pack=b2pack, inv1=inv1, inv2=inv2,
                bias1p=bias1p.astype(np.float32), bias2p=bias2p.astype(np.float32))


# ---------------------------------------------------------------- device
def build_program(plan, wp, ncore):
    ncap, nt, totc, gbase = plan["ncap"], plan["nt"], plan["totc"], plan["gbase"]
    tiles = plan["tiles"]
    ng = plan["ng"]
    patv, patoffs = _patterns()
    edv, edoffs = _repmats()
    npat = patv.shape[1]

    nc = bacc.Bacc("TRN2", num_swdge_queues=4)
    XT = nc.declare_dram_parameter("XT", [128, ncap], F16, isOutput=False)
    W1P = nc.declare_dram_parameter("W1P", [128, 130], F16, isOutput=False)
    B1P = nc.declare_dram_parameter("B1P", [128, 130], F16, isOutput=False)
    W2P = nc.declare_dram_parameter("W2P", [64, 34], F16, isOutput=False)
    B2P = nc.declare_dram_parameter("B2P", [128, 34], F16, isOutput=False)
    IDX = nc.declare_dram_parameter("IDX", [128, totc * 8], mybir.dt.int16, isOutput=False)
    MASK = nc.declare_dram_parameter("MASK", [128, totc], F16, isOutput=False)
    PATS = nc.declare_dram_parameter("PATS", [128, npat], F16, isOutput=False)
    EDS = nc.declare_dram_parameter("EDS", [128, len(BUCKETS) * 128], F16, isOutput=False)
    VPERM = nc.declare_dram_parameter("VPERM", [128, len(BUCKETS) * 128], F16, isOutput=False)
    EYE = nc.declare_dram_parameter("EYE", [128, 128], F16, isOutput=False)
    INV1 = nc.declare_dram_parameter("INV1", [128, 64], F32, isOutput=False)
    BS1 = nc.declare_dram_parameter("BS1", [128, 64], F32, isOutput=False)
    INV2 = nc.declare_dram_parameter("INV2", [128, 16], F32, isOutput=False)
    BS2 = nc.declare_dram_parameter("BS2", [128, 16], F32, isOutput=False)
    OUT = nc.declare_dram_parameter("OUT", [ncap, 16], F32, isOutput=True)

    T1s = nc.dram_tensor("T1s", [ncap, 128], F16)
    shared = "Shared" if ncore > 4 else "Local"
    T1f = nc.dram_tensor("T1f", [ng, 128], F16, addr_space=shared)
    T2s = nc.dram_tensor("T2s", [ncap, 128], F16)
    T2f = nc.dram_tensor("T2f", [ng, 128], F16, addr_space=shared)
    # Local copies of the gathered tables: SWDGE random reads from the
    # Shared address space run ~3x slower than from Local DRAM, so copy
    # once (sequential, cheap) and gather from the Local replica.
    T1fl = nc.dram_tensor("T1fl", [ng, 128], F16)
    T2fl = nc.dram_tensor("T2fl", [ng, 128], F16)

    with TileContext(nc) as tc:
        nc.gpsimd.load_library(mlp)
        with tc.tile_pool(name="const", bufs=1) as cpool, \
             tc.tile_pool(name="work", bufs=6) as pool, \
             tc.tile_pool(name="zpool", bufs=7) as zpool, \
             tc.tile_pool(name="pz", bufs=3, space="PSUM") as pzpool, \
             tc.tile_pool(name="pa", bufs=2, space="PSUM") as papool:

            # persistent constants
            w1p = cpool.tile([128, 130], F16); nc.sync.dma_start(w1p[:], W1P[:])
            b1p = cpool.tile([128, 130], F16); nc.sync.dma_start(b1p[:], B1P[:])
            w2p = cpool.tile([64, 34], F16);   nc.sync.dma_start(w2p[:], W2P[:])
            b2p = cpool.tile([128, 34], F16);  nc.sync.dma_start(b2p[:], B2P[:])
            idxs = cpool.tile([128, totc * 8], mybir.dt.int16)
            nc.sync.dma_start(idxs[:], IDX[:])
            maskt = cpool.tile([128, totc], F16); nc.sync.dma_start(maskt[:], MASK[:])
            pats = cpool.tile([128, npat], F16); nc.sync.dma_start(pats[:], PATS[:])
            eds = cpool.tile([128, len(BUCKETS) * 128], F16)
            nc.sync.dma_start(eds[:], EDS[:])
            vperm = cpool.tile([128, len(BUCKETS) * 128], F16)
            nc.sync.dma_start(vperm[:], VPERM[:])
            eye = cpool.tile([128, 128], F16); nc.sync.dma_start(eye[:], EYE[:])
            inv1 = cpool.tile([128, 64], F32); nc.sync.dma_start(inv1[:], INV1[:])
            bs1 = cpool.tile([128, 64], F32); nc.sync.dma_start(bs1[:], BS1[:])
            inv2 = cpool.tile([128, 16], F32); nc.sync.dma_start(inv2[:], INV2[:])
            bs2 = cpool.tile([128, 16], F32); nc.sync.dma_start(bs2[:], BS2[:])
            xrt = cpool.tile([128, nt * 65], F16)      # x_r''' per sorted node
            h2rt = cpool.tile([128, nt * 17], F16)     # layer-2 r-side per node
            xs2 = [cpool.tile([128, 32 * 65], F16, name=f"xs{i}")
                   for i in range(4)]
            stg2 = [cpool.tile([128, 128], F16, name=f"stg{i}")
                    for i in range(3)]
            for t_ in xs2 + stg2:
                nc.gpsimd.memset(t_[:], 0.0)
            nc.gpsimd.memset(xrt[:], 0.0)
            nc.gpsimd.memset(h2rt[:], 0.0)

            # ---------------- phase A: layer-1 tables ----------------
            xrt_w, h2rt_w = [], []
            ni_regs = {cc: nc.gpsimd.to_reg(cc * 128)
                       for cc in sorted({min(b, 32) for b in BUCKETS})}
            for t in range(nt):
                xtc = pool.tile([128, 128], F16, tag="xtc")
                nc.sync.dma_start(xtc[:], XT[:, t * 128:(t + 1) * 128])
                psA = papool.tile([128, 130], F32, tag="tmp")
                stg = stg2[t % 3]
                nc.tensor.matmul(psA[:], xtc[:], w1p[:], start=True, stop=True)
                nc.vector.tensor_add(stg[:, 0:65], psA[:, 0:65], b1p[:, 0:65])
                nc.vector.tensor_scalar(stg[:, 65:66], psA[:, 64:65], 0.0, 1.0,
                                        mybir.AluOpType.mult, mybir.AluOpType.add)
                xrt_w.append(nc.vector.tensor_add(
                    xrt[:, t * 65:(t + 1) * 65],
                    psA[:, 65:130], b1p[:, 65:130]).ins)
                nc.sync.dma_start(T1s[t * 128:(t + 1) * 128, :], stg[:])
            if not os.environ.get("GAT_SKIP_CC"):
                nc.gpsimd.collective_compute(
                    "AllGather", mybir.AluOpType.bypass,
                    replica_groups=[list(range(ncore))],
                    ins=[T1s[:]], outs=[T1f[:]])
            else:
                nc.sync.dma_start(T1f[0:ncap, :], T1s[:, :])
            hg = ng // 2
            nc.sync.dma_start(T1fl[0:hg, :], T1f[0:hg, :])
            nc.scalar.dma_start(T1fl[hg:, :], T1f[hg:, :])

            # ---------------- phase C/E: per-layer edge phases ----------------
            qrr = [0]  # SWDGE queue round-robin

            def layer(F, Tf, xr_src, xr_w, n_pos, emit, xr_dep=None):
                K = 4      # pass2 (softmax+agg) lags pass1 by K subs

                def pass2(w):
                    C, b, prow = w["C"], w["b"], w["prow"]
                    az, pm, zt, psa = w["az"], w["pm"], w["zt"], w["psa"]
                    rp = pool.tile([128, 32], F32, tag="rp")
                    rm = pool.tile([128, 32], F32, tag="rm")
                    nc.vector.reduce_sum(rp[:, 0:C], az[:, 0:C, 0:n_pos],
                                         axis=AX.X)
                    nc.vector.reduce_sum(rm[:, 0:C], az[:, 0:C, n_pos:F],
                                         axis=AX.X)
                    u = pool.tile([128, 32], F32, tag="u")
                    nc.vector.tensor_sub(u[:, 0:C], rp[:, 0:C], rm[:, 0:C])
                    nc.vector.tensor_add(u[:, 0:C], u[:, 0:C], pm[:, 0:C])
                    ex = pool.tile([128, 32], F16, tag="ex")
                    nc.scalar.activation(ex[:, 0:C], u[:, 0:C], AF.Exp,
                                         scale=0.4)
                    sv = pool.tile([128, 32 * 32], F16, tag="sv")
                    kd = (32 * b) // 128      # chunks per 32-node block
                    nblk = C // kd
                    pf = pats[:, :]
                    pat_ap = bass.AP(pf.tensor, pf.offset + patoffs[b],
                                     [[pf.ap[0][0], 128], [0, nblk],
                                      [1, kd * 32]])
                    svv = sv.rearrange("p (n m) -> p n m", m=kd * 32)[:, 0:nblk, :]
                    exv = ex.rearrange("p (n k) -> p n k", k=kd)[:, 0:nblk, :]
                    exb = exv.unsqueeze(3).broadcast_to([128, nblk, kd, 32])
                    nc.vector.tensor_mul(
                        svv.rearrange("p n (k m) -> p n k m", m=32), pat_ap, exb)
                    for c in range(C):
                        blk = prow // 32 + c // kd
                        nc.tensor.matmul(
                            psa[32 * blk:32 * blk + 32, :],
                            sv[:, c * 32:(c + 1) * 32],
                            zt[:, c, 0:F + 2],
                            start=(c % kd == 0), stop=(c % kd == kd - 1),
                            tile_position=(0, 32 * blk), skip_group_check=True)
                    if w["last"]:
                        emit(w["ti"], psa)

                pend = []
                soff = 0   # chunk offset
                for ti, (b, p0) in enumerate(tiles):
                    xs = xs2[ti % 4]
                    subs = [(0, b)] if b <= 32 else [(0, 32), (64, 32)]
                    psa = papool.tile([128, F + 2], F32, tag="psa", bufs=3)
                    for si, (prow, C) in enumerate(subs):
                        zt = zpool.tile([128, 32, 128], F16, tag="zt")
                        if os.environ.get("GAT_SKIP_GATHER"):
                            nc.sync.dma_start(zt[:, 0:C, :],
                                              Tf[0:128, :].unsqueeze(1).broadcast_to([128, C, 128]))
                        else:
                            GMAX = 4 if os.environ.get("GAT_GMAX4") else 8  # <=1024 idxs (ucode limit)
                            for g0 in range(0, C, GMAX):
                                g1 = min(g0 + GMAX, C)
                                nig = (g1 - g0) * 128
                                nc.gpsimd.dma_gather(
                                    zt[:, g0:g1, :], Tf[gbase:, :],
                                    idxs[:, (soff + g0) * 8:(soff + g1) * 8],
                                    nig, nig, 128, queue_num=qrr[0] % 4)
                                qrr[0] += 1
                        # spread xr rows: node j of chunk c at partition j*b.
                        # For b>=16 the table is qperm-ordered so slot-j's
                        # nodes sit at contiguous rows [j*C, (j+1)*C).
                        npchunk = 128 // b
                        xsf = xs[:, :]
                        sps = xsf.ap[0][0]
                        if b >= 16:
                            for j in range(npchunk):
                                src = xr_src[prow + j * C:prow + (j + 1) * C,
                                             ti * (F + 1):(ti + 1) * (F + 1)]
                                dst = xs[j * b:j * b + 1, 0:C * (F + 1)]
                                eng = nc.sync if j % 2 == 0 else nc.scalar
                                iv = eng.dma_start(dst, src)
                                if xr_dep is not None:
                                    add_dep_helper(iv.ins, xr_dep[ti], sync=True,
                                                   reason="spread reads xr table")
                        else:
                            for c in range(C):
                                src = xr_src[prow + c * npchunk:prow + (c + 1) * npchunk,
                                             ti * (F + 1):(ti + 1) * (F + 1)]
                                dst = bass.AP(xsf.tensor, xsf.offset + c * (F + 1),
                                              [[sps * b, npchunk], [1, F + 1]])
                                eng = nc.sync if c % 2 == 0 else nc.scalar
                                iv = eng.dma_start(dst, src)
                                if xr_dep is not None:
                                    add_dep_helper(iv.ins, xr_dep[ti], sync=True,
                                                   reason="spread reads xr table")
                        # z' psum: vals + q separately (bank-aligned)
                        pzq = papool.tile([128, 32], F32, tag="tmp")
                        cpg = 512 // F
                        xsv = xs[:, 0:C * (F + 1)].rearrange("p (c f) -> p c f", f=F + 1)
                        az = pool.tile([128, 32, F], F16, tag="az")
                        for c0 in range(0, C, cpg):
                            c1 = min(c0 + cpg, C)
                            pz = pzpool.tile([128, cpg * F], F32, tag="pz")
                            nc.tensor.matmul(pz[:, 0:(c1 - c0) * F], eye[:],
                                             zt[:, c0:c1, 0:F],
                                             start=True, stop=False)
                            nc.tensor.matmul(pz[:, 0:(c1 - c0) * F],
                                             eds[:, edoffs[b]:edoffs[b] + 128],
                                             xsv[:, c0:c1, 0:F],
                                             start=False, stop=True)
                            pzv = pz.rearrange("p (c f) -> p c f", f=F)[:, 0:c1 - c0, :]
                            nc.scalar.activation(az[:, c0:c1, :], pzv[:, :, :], AF.Abs)
                        ztf = zt[:, :, :]
                        zqcol = bass.AP(ztf.tensor, ztf.offset + F,
                                        [[ztf.ap[0][0], 128], [128, C]])
                        nc.tensor.matmul(pzq[:, 0:C], eye[:], zqcol,
                                         start=True, stop=False)
                        xqcol = bass.AP(xsf.tensor, xsf.offset + F,
                                        [[sps, 128], [F + 1, C]])
                        nc.tensor.matmul(pzq[:, 0:C],
                                         eds[:, edoffs[b]:edoffs[b] + 128],
                                         xqcol, start=False, stop=True)
                        pm = pool.tile([128, 32], F32, tag="pm")
                        nc.vector.tensor_add(pm[:, 0:C], pzq[:, 0:C],
                                             maskt[:, soff:soff + C])
                        pend.append(dict(C=C, b=b, prow=prow, az=az, pm=pm,
                                         zt=zt, psa=psa, ti=ti,
                                         last=(si == len(subs) - 1)))
                        soff += C
                        if len(pend) > K:
                            pass2(pend.pop(0))
                for w in pend:
                    pass2(w)

            # layer-1 epilogue: h, transpose, layer-2 tables
            def emit1(ti, psa):
                stg = stg2[ti % 3]
                rden = pool.tile([128, 1], F32, tag="rden")
                nc.vector.reciprocal(rden[:], psa[:, 65:66])
                h1 = pool.tile([128, 64], F32, tag="h1")
                nc.vector.tensor_scalar_mul(h1[:], psa[:, 0:64], rden[:])
                nc.vector.tensor_mul(h1[:], h1[:], inv1[:])
                nc.vector.tensor_add(h1[:], h1[:], bs1[:])
                h = pool.tile([128, 64], F16, tag="h")
                nc.scalar.activation(h[:], h1[:], AF.Relu)
                ptp = papool.tile([64, 128], F16, tag="tmp")
                bt = tiles[ti][0]
                nc.tensor.transpose(ptp[:], h[:],
                                    vperm[:, edoffs[bt]:edoffs[bt] + 128])
                hT = pool.tile([64, 128], F16, tag="hT")
                nc.scalar.copy(hT[:], ptp[:])
                ps2 = papool.tile([128, 34], F32, tag="tmp")
                nc.tensor.matmul(ps2[:], hT[:], w2p[:], start=True, stop=True)
                nc.vector.tensor_add(stg[:, 0:17], ps2[:, 0:17], b2p[:, 0:17])
                nc.vector.tensor_scalar(stg[:, 17:18], ps2[:, 16:17], 0.0, 1.0,
                                        mybir.AluOpType.mult, mybir.AluOpType.add)
                h2rt_w.append(nc.vector.tensor_add(
                    h2rt[:, ti * 17:(ti + 1) * 17],
                    ps2[:, 17:34], b2p[:, 17:34]).ins)
                nc.sync.dma_start(T2s[ti * 128:(ti + 1) * 128, :], stg[:, :])

            def emit2(ti, psa):
                rden = pool.tile([128, 1], F32, tag="rden")
                nc.vector.reciprocal(rden[:], psa[:, 17:18])
                o1 = pool.tile([128, 16], F32, tag="o1")
                nc.vector.tensor_scalar_mul(o1[:], psa[:, 0:16], rden[:])
                nc.vector.tensor_mul(o1[:], o1[:], inv2[:])
                nc.vector.tensor_add(o1[:], o1[:], bs2[:])
                nc.sync.dma_start(OUT[ti * 128:(ti + 1) * 128, :], o1[:])

            layer(64, T1fl, xrt, 65, wp["n1p"], emit1, xr_dep=xrt_w)
            if not os.environ.get("GAT_SKIP_CC"):
                nc.gpsimd.collective_compute(
                    "AllGather", mybir.AluOpType.bypass,
                    replica_groups=[list(range(ncore))],
                    ins=[T2s[:]], outs=[T2f[:]])
            else:
                nc.sync.dma_start(T2f[0:ncap, :], T2s[:, :])
            nc.sync.dma_start(T2fl[0:hg, :], T2f[0:hg, :])
            nc.scalar.dma_start(T2fl[hg:, :], T2f[hg:, :])
            layer(16, T2fl, h2rt, 17, wp["n2p"], emit2, xr_dep=h2rt_w)

    nc.compile()
    return nc


# ---------------------------------------------------------------- host entry
def kernel(x, edge_index, W1_l, W1_r, b1_l, b1_r, a1, bias1,
           W2_l, W2_r, b2_l, b2_r, a2, bias2, _run=None, _ncore=NCORE):
    x = np.asarray(x, np.float32)
    ei = np.asarray(edge_index)
    n = x.shape[0]
    loop = np.arange(n, dtype=ei.dtype)
    src = np.concatenate([np.asarray(ei[0]), loop]).astype(np.int64)
    dst = np.concatenate([np.asarray(ei[1]), loop]).astype(np.int64)

    plan = build_plan(src, dst, n, _ncore)
    wp = prep_weights(*[np.asarray(a, np.float32) for a in
                        (W1_l, W1_r, b1_l, b1_r, a1, bias1,
                         W2_l, W2_r, b2_l, b2_r, a2, bias2)])
    nc = build_program(plan, wp, _ncore)

    patv, _ = _patterns()
    edvv, _ = _repmats()
    vmv = _vmats()
    in_maps = []
    for c in range(_ncore):
        xt = np.zeros((128, plan["ncap"]), np.float16)
        ordc = plan["order"][c]
        valid = ordc >= 0
        xt[:, plan["colmap"][np.where(valid)[0]]] = \
            x[ordc[valid]].T.astype(np.float16)
        in_maps.append({
            "XT": xt,
            "W1P": wp["w1pack"].astype(np.float16),
            "B1P": np.tile(wp["b1pack"][None, :].astype(np.float16), (128, 1)),
            "W2P": wp["w2pack"].astype(np.float16),
            "B2P": np.tile(wp["b2pack"][None, :].astype(np.float16), (128, 1)),
            "IDX": plan["idxw"][c],
            "MASK": plan["mask"][c],
            "PATS": patv,
            "EDS": edvv,
            "VPERM": vmv,
            "EYE": np.eye(128, dtype=np.float16),
            "INV1": np.tile(wp["inv1"][None, :], (128, 1)),
            "BS1": np.tile(wp["bias1p"][None, :], (128, 1)),
            "INV2": np.tile(wp["inv2"][None, :], (128, 1)),
            "BS2": np.tile(wp["bias2p"][None, :], (128, 1)),
        })

    if _run is None:
        import time as _time
        res = run_bass_kernel_spmd(nc, in_maps, list(range(_ncore)))
        outs = [r["OUT"] for r in res.results]
        if os.environ.get("GAT_TRACE"):
            ts = []
            for _ in range(3):
                t0 = _time.time()
                run_bass_kernel_spmd(nc, in_maps, list(range(_ncore)))
                ts.append(_time.time() - t0)
            # min wall of a cached re-dispatch (includes host<->device I/O)
            print(f"HW exec time: {int(min(ts) * 1e9)} ns (e2e dispatch wall, "
                  f"runs: {[f'{t:.3f}s' for t in ts]})")
    else:
        outs = _run(nc, in_maps)   # test hook: returns list of OUT per core

    # unshard: rows sorted-order per core -> natural; cols: undo p2
    out = np.zeros((n, 16), np.float32)
    for c in range(_ncore):
        ordc = plan["order"][c]
        valid = ordc >= 0
        out[ordc[valid]] = outs[c][np.where(valid)[0]]
    inv_p2 = np.argsort(wp["p2"])
    return out[:, inv_p2].astype(np.float32)

